# revision 5
# baseline (speedup 1.0000x reference)
"""GraphUnetNoPool (7-layer GCN U-net, no pooling) on 8 trn2 NeuronCores.

Math: gn = D^-1/2 (g+I) D^-1/2;  layer: h' = relu(gn @ h @ W.T + b)
Rewrite: u = dinv*h;  v = (g+I) @ u;  h' = relu((dinv*v) @ W.T + b)
  =>  per-core row-slab m:  v.T[d, m] = sum_k u[k, d] * A[k, m]  (A symmetric:
      column slab of A == transposed row slab, so lhsT = u natural layout and
      rhs = A[:, slab] streams naturally from DRAM rows).
Sharding: 1D row-parallel. Core c owns rows [c*S, (c+1)*S).

v2: mm1 in single bf16 (u rounded to bf16; A exact in fp8/bf16), A held
fully resident in SBUF as fp8 (entries {0,1,2} are exact), mm2/weights in
bf16, and the per-layer AllGather of u split into G groups, each issued as
soon as its row block is ready so the collective overlaps the next layer's
mm1 (which consumes gathered groups in arrival order).
"""

import numpy as np
from contextlib import ExitStack

import concourse.bass as bass
import concourse.tile as tile
from concourse import bacc, mybir
from concourse.bass_utils import run_bass_kernel_spmd
from concourse.masks import make_identity

F32 = mybir.dt.float32
BF16 = mybir.dt.bfloat16
F8 = mybir.dt.float8e4

N, D, C, L = 8192, 256, 8, 7
S = N // C            # 1024 rows per core
KC = N // 128         # 64 k-chunks
MQ = S // 128         # 8 m-chunks per slab
G_AG = 2              # allgather split groups
A_DT = F8             # resident adjacency dtype ({0,1,2} exact in fp8e4)


def build_nc(n=N, d=D, c=C, n_layers=L, repeat=1, g_ag=G_AG, a_dt=A_DT,
             mm1_f8=MM1_F8):
    s = n // c
    kc = n // 128
    mq = s // 128
    nmh = s // 512 if s >= 512 else 1   # moving halves of 512 (psum bank)
    mw = min(s, 512)                    # moving width
    dh_n = d // 128                     # d chunks (2 for d=256)
    G = g_ag
    sgk = (s // G) // 128               # k-chunks per (core, group)
    gm = mq // G                        # m-chunks per group
    gk = kc // G                        # k-chunks per group (all cores)
    assert d % 128 == 0 and s % (128 * G) == 0
    if mm1_f8:
        assert a_dt == F8 and sgk % 2 == 0
    u_dt = F8 if mm1_f8 else BF16
    ud_n = 2 if mm1_f8 else 1           # u planes (hi, scaled lo)

    nc = bacc.Bacc("TRN2", target_bir_lowering=False, debug=False, num_devices=c)

    a_dram = nc.dram_tensor("a_slab", [n, s], F32, kind="ExternalInput")
    u0_dram = nc.dram_tensor("u0", [n, ud_n * d], u_dt, kind="ExternalInput")
    h0s_dram = nc.dram_tensor("h0_slab", [s, d], F32, kind="ExternalInput")
    dslab_dram = nc.dram_tensor("dinv_slab", [128, mq], F32, kind="ExternalInput")
    dbc_dram = nc.dram_tensor("dinv_bcast", [128, s], F32, kind="ExternalInput")
    wt_dram = nc.dram_tensor("wt", [n_layers, d, d], BF16, kind="ExternalInput")
    bias_dram = nc.dram_tensor("bias_t", [128, 2 * n_layers], F32, kind="ExternalInput")
    out_dram = nc.dram_tensor("out", [4, s, d], F32, kind="ExternalOutput")

    with ExitStack() as ctx:
        tc = ctx.enter_context(tile.TileContext(nc))
        dram = ctx.enter_context(tc.tile_pool(name="dram", bufs=1, space="DRAM"))
        res = ctx.enter_context(tc.tile_pool(name="res", bufs=1))
        stage = ctx.enter_context(tc.tile_pool(name="stage", bufs=2))
        up = ctx.enter_context(tc.tile_pool(name="up", bufs=1))
        wtp = ctx.enter_context(tc.tile_pool(name="wtp", bufs=2))
        work = ctx.enter_context(tc.tile_pool(name="work", bufs=2))
        slabp = ctx.enter_context(tc.tile_pool(name="slabp", bufs=2))
        pmm1 = ctx.enter_context(tc.tile_pool(name="pmm1", bufs=4, space="PSUM"))
        post = ctx.enter_context(tc.tile_pool(name="post", bufs=4, space="PSUM"))

        # ---- persistent DRAM scratch ----
        ag_ins = [
            dram.tile([s // G, d], BF16, name=f"ag_in{j}", tag=f"ag_in{j}", bufs=2)
            for j in range(G)
        ]
        ag_outs = [
            [
                dram.tile(
                    [(n // G), d], BF16, name=f"ag_out{i}_{j}", tag=f"ag_out{i}_{j}",
                    addr_space="Shared",
                )
                for j in range(G)
            ]
            for i in range((n_layers - 1) * repeat)
        ]
        skip_dram = dram.tile([3, s, d], F32, name="skip_dram")

        # ---- persistent SBUF ----
        a_sb = res.tile([128, kc, s], a_dt, name="a_sb")
        dinv_sb = res.tile([128, mq], F32, name="dinv_sb")
        dinv_bc = res.tile([128, s], F32, name="dinv_bc")
        bias_sb = res.tile([128, 2 * n_layers], F32, name="bias_sb")
        ident = res.tile([128, 128], F32, name="ident")

        make_identity(nc, ident)
        nc.sync.dma_start(out=dinv_sb, in_=dslab_dram[:, :])
        nc.sync.dma_start(out=dinv_bc, in_=dbc_dram[:, :])
        nc.sync.dma_start(out=bias_sb, in_=bias_dram[:, :])

        # ---- startup: load A column-slab, cast to a_dt, fully resident ----
        for k in range(kc):
            st = stage.tile([128, s], F32, name="st", tag="stage")
            nc.sync.dma_start(out=st, in_=a_dram[k * 128 : (k + 1) * 128, :])
            nc.vector.tensor_copy(a_sb[:, k, :], st)

        relu = mybir.ActivationFunctionType.Relu
        skip_slot = {4: 2, 5: 1, 6: 0}  # up-layer l uses skip h_{...} slot

        # u0 grouped view: [ci, g, p, kk, d] (per-ci DMAs at layer 0)
        u0_g = u0_dram[:, :].rearrange(
            "(ci g kk p) d2 -> ci g p kk d2", g=G, kk=sgk, p=128
        )

        for rep_l in range(n_layers * repeat):
            rep, l = divmod(rep_l, n_layers)
            # ---- Phase A: load U per gather group ----
            u_g = [
                up.tile([128, c, sgk, d], BF16, name=f"u_g{j}", tag=f"u_g{j}")
                for j in range(G)
            ]
            for j in range(G):
                if l == 0:
                    for ci in range(c):
                        nc.sync.dma_start(out=u_g[j][:, ci], in_=u0_g[ci, j])
                else:
                    src = ag_outs[rep * (n_layers - 1) + l - 1][j]
                    # rows are (ci kk)-major: mergeable to one 32-chunk dim
                    nc.sync.dma_start(
                        out=u_g[j],
                        in_=src.rearrange("(t p) d2 -> p t d2", p=128),
                    )

            # per-layer weight prefetch (bf16)
            wt_t = wtp.tile([128, dh_n, d], BF16, name="wt_t", tag="wt")
            nc.sync.dma_start(
                out=wt_t, in_=wt_dram[l].rearrange("(kc p) o -> p kc o", p=128)
            )

            # skip-connection preload for NEXT layer's input (scaled by dinv)
            nl = l + 1
            skip_sb = None
            if nl in skip_slot and nl < n_layers:
                skip_sb = slabp.tile([128, mq, d], F32, name="skip_sb", tag="skip")
                nc.sync.dma_start(
                    out=skip_sb,
                    in_=skip_dram[skip_slot[nl]].rearrange(
                        "(m p) d2 -> p m d2", p=128
                    ),
                )
                for m in range(mq):
                    nc.vector.tensor_scalar(
                        out=skip_sb[:, m, :],
                        in0=skip_sb[:, m, :],
                        scalar1=dinv_sb[:, m : m + 1],
                        scalar2=None,
                        op0=mybir.AluOpType.mult,
                    )

            # ---- Phase B: mm1  v.T[d, m] accumulate over k (bf16 single) ----
            psv = [
                [pmm1.tile([128, mw], F32, name="psv", tag="pmm1") for _ in range(nmh)]
                for _ in range(dh_n)
            ]
            v_sb = [
                work.tile([128, s], BF16, name="v_sb", tag="vsb") for _ in range(dh_n)
            ]
            for mh in range(nmh):
                msl = slice(mh * mw, (mh + 1) * mw)
                for j in range(G):
                    for t in range(gk):
                        ci, kk = divmod(t, sgk)
                        k = ci * (s // 128) + j * sgk + kk
                        rhs = a_sb[:, k, msl]
                        for dh in range(dh_n):
                            nc.tensor.matmul(
                                psv[dh][mh],
                                u_g[j][:, ci, kk, dh * 128 : (dh + 1) * 128],
                                rhs,
                                start=(j == 0 and t == 0),
                                stop=(j == G - 1 and t == gk - 1),
                            )
                # ---- Phase C (per half): copy v.T to SBUF (bf16 for mm2) ----
                for dh in range(dh_n):
                    nc.vector.tensor_copy(v_sb[dh][:, msl], psv[dh][mh])

            # ---- Phase D/E per m-half: mm2, relu, transpose, u-prep ----
            is_out = l >= n_layers - 3  # layers 4,5,6 emit outputs 0,1,2
            save_skip = l <= 2
            h_nat = None
            if is_out or save_skip:
                h_nat = slabp.tile([128, mq, d], F32, name="h_nat", tag="hnat", bufs=1)
            if l == n_layers - 1:
                # reuse the (now idle) skip/us tags for the final-layer tiles
                h0s = slabp.tile([128, mq, d], F32, name="h0s", tag="skip")
                nc.sync.dma_start(
                    out=h0s, in_=h0s_dram[:, :].rearrange("(m p) d2 -> p m d2", p=128)
                )
                out3 = slabp.tile([128, mq, d], F32, name="out3", tag="us_hi", bufs=1)
            if l < n_layers - 1:
                us_hi = slabp.tile(
                    [128, mq, d], BF16, name="us_hi", tag="us_hi", bufs=1
                )

            hT = [work.tile([128, s], F32, name="hT", tag="hT") for _ in range(dh_n)]
            mq_h = mw // 128  # m-chunks per half
            for mh in range(nmh):
                msl = slice(mh * mw, (mh + 1) * mw)
                pso = [
                    post.tile([128, mw], F32, name="pso", tag="post")
                    for _ in range(dh_n)
                ]
                for dho in range(dh_n):
                    for kin in range(dh_n):
                        nc.tensor.matmul(
                            pso[dho],
                            wt_t[:, kin, dho * 128 : (dho + 1) * 128],
                            v_sb[kin][:, msl],
                            start=(kin == 0),
                            stop=(kin == dh_n - 1),
                        )
                for dho in range(dh_n):
                    nc.vector.tensor_mul(hT[dho][:, msl], pso[dho], dinv_bc[:, msl])
                    nc.scalar.activation(
                        hT[dho][:, msl],
                        hT[dho][:, msl],
                        relu,
                        bias=bias_sb[:, 2 * l + dho : 2 * l + dho + 1],
                    )
                for m in range(mh * mq_h, (mh + 1) * mq_h):
                    tp = post.tile([128, d], F32, name="tp", tag="post")
                    for dh in range(dh_n):
                        nc.tensor.transpose(
                            tp[:, dh * 128 : (dh + 1) * 128],
                            hT[dh][:, m * 128 : (m + 1) * 128],
                            ident,
                        )
                    if l < n_layers - 1:
                        ufp = stage.tile([128, d], F32, name="ufp", tag="ufp")
                        dv = dinv_sb[:, m : m + 1]
                        if skip_sb is not None:
                            nc.vector.scalar_tensor_tensor(
                                out=ufp,
                                in0=tp,
                                scalar=dv,
                                in1=skip_sb[:, m, :],
                                op0=mybir.AluOpType.mult,
                                op1=mybir.AluOpType.add,
                            )
                        else:
                            nc.vector.tensor_scalar(
                                out=ufp,
                                in0=tp,
                                scalar1=dv,
                                scalar2=None,
                                op0=mybir.AluOpType.mult,
                            )
                        nc.vector.tensor_copy(us_hi[:, m, :], ufp)
                    if h_nat is not None:
                        nc.scalar.copy(h_nat[:, m, :], tp)
                    if l == n_layers - 1:
                        nc.vector.tensor_add(out3[:, m, :], tp, h0s[:, m, :])
                    # group complete -> stage + allgather immediately
                    if l < n_layers - 1 and (m + 1) % gm == 0:
                        j = m // gm
                        agi = ag_ins[j]
                        nc.sync.dma_start(
                            out=agi.rearrange("(mm p) d2 -> p mm d2", p=128),
                            in_=us_hi[:, j * gm : (j + 1) * gm, :],
                        )
                        nc.gpsimd.collective_compute(
                            "AllGather",
                            mybir.AluOpType.bypass,
                            replica_groups=[list(range(c))],
                            ins=[agi.opt()],
                            outs=[ag_outs[rep * (n_layers - 1) + l][j].opt()],
                        )

            # ---- Phase F: DMAs out ----
            if save_skip:
                nc.sync.dma_start(
                    out=skip_dram[l].rearrange("(m p) d2 -> p m d2", p=128),
                    in_=h_nat,
                )
            if is_out:
                nc.sync.dma_start(
                    out=out_dram[l - (n_layers - 3)].rearrange(
                        "(m p) d2 -> p m d2", p=128
                    ),
                    in_=h_nat,
                )
            if l == n_layers - 1:
                nc.sync.dma_start(
                    out=out_dram[3].rearrange("(m p) d2 -> p m d2", p=128), in_=out3
                )

    nc.compile()
    return nc


try:
    import ml_dtypes

    ml_bf16 = ml_dtypes.bfloat16
except ImportError:  # pragma: no cover
    import jax.numpy as jnp

    ml_bf16 = jnp.bfloat16


def prep_inputs(g, h, W_down, b_down, W_bottom, b_bottom, W_up, b_up, c=C):
    """Host-side sharding + layout prep. Returns per-core input maps."""
    n = g.shape[0]
    s = n // c
    d = h.shape[1]
    g = np.asarray(g, np.float32)
    h = np.asarray(h, np.float32)
    deg = g.sum(axis=1) + 1.0
    dinv = (1.0 / np.sqrt(deg)).astype(np.float32)

    u0 = (h * dinv[:, None]).astype(np.float32)
    u0_packed = np.asarray(u0.astype(ml_bf16))  # [n, d] bf16

    Ws = [W_down[0], W_down[1], W_down[2], W_bottom, W_up[0], W_up[1], W_up[2]]
    bs = [b_down[0], b_down[1], b_down[2], b_bottom, b_up[0], b_up[1], b_up[2]]
    wt = np.stack(
        [np.ascontiguousarray(np.asarray(W, np.float32).T).astype(ml_bf16) for W in Ws]
    )
    nl = len(Ws)
    bias_t = np.zeros((128, 2 * nl), np.float32)
    for li, b in enumerate(bs):
        b = np.asarray(b, np.float32)
        for dh in range(d // 128):
            bias_t[:, 2 * li + dh] = b[dh * 128 : (dh + 1) * 128]

    in_maps = []
    for ci in range(c):
        sl = slice(ci * s, (ci + 1) * s)
        a_slab = np.ascontiguousarray(g[:, sl])
        idx = np.arange(s)
        a_slab[ci * s + idx, idx] += 1.0  # fold self-loops into the slab
        dinv_slab = dinv[sl].reshape(s // 128, 128).T.copy()  # [128, mq]
        dinv_bcast = np.broadcast_to(dinv[sl][None, :], (128, s)).copy()
        in_maps.append(
            dict(
                a_slab=a_slab,
                u0=u0_packed,
                h0_slab=np.ascontiguousarray(h[sl]),
                dinv_slab=dinv_slab,
                dinv_bcast=dinv_bcast,
                wt=wt,
                bias_t=bias_t,
            )
        )
    return in_maps


_NC_CACHE = {}


def kernel(g, h, W_down, b_down, W_bottom, b_bottom, W_up, b_up):
    key = "full"
    if key not in _NC_CACHE:
        _NC_CACHE[key] = build_nc()
    nc = _NC_CACHE[key]
    in_maps = prep_inputs(g, h, W_down, b_down, W_bottom, b_bottom, W_up, b_up)
    res = run_bass_kernel_spmd(nc, in_maps, list(range(C)))
    outs = [np.asarray(r["out"]).reshape(4, S, D) for r in res.results]
    full = np.concatenate(outs, axis=1)  # [4, N, D]
    return full.astype(np.float32)


if __name__ == "__main__":
    import reference

    inputs = reference.setup_inputs()
    inputs = {k: np.asarray(v) for k, v in inputs.items()}
    out = kernel(**inputs)
    exp = np.asarray(reference.reference(**reference.setup_inputs()))
    err = np.abs(out - exp).max() / (np.abs(exp).max() + 1e-30)
    rel = np.linalg.norm(out - exp) / (np.linalg.norm(exp) + 1e-30)
    print("max-scaled err:", err, "rel l2:", rel)


# revision 7
# speedup vs baseline: 1.7912x; 1.7912x over previous
"""GraphUnetNoPool (7-layer GCN U-net, no pooling) on 8 trn2 NeuronCores.

Math: gn = D^-1/2 (g+I) D^-1/2;  layer: h' = relu(gn @ h @ W.T + b)
Rewrite: u = dinv*h;  v = (g+I) @ u;  h' = relu((dinv*v) @ W.T + b)
  =>  per-core row-slab m:  v.T[d, m] = sum_k u[k, d] * A[k, m]  (A symmetric:
      column slab of A == transposed row slab, so lhsT = u natural layout and
      rhs = A[:, slab] streams naturally from DRAM rows).
Sharding: 1D row-parallel. Core c owns rows [c*S, (c+1)*S).

v2: mm1 in single bf16 (u rounded to bf16; A exact in fp8), A fully resident
in SBUF as fp8, mm2/weights in bf16, per-layer AllGather of u split into G
groups, each issued as soon as its row block is ready so the collective
overlaps the next layer's mm1 (which consumes gathered groups in order).
v3 (mm1_f8): u as fp8 hi + 16*lo pair, mm1 via DoubleRow fp8 matmuls (2
k-chunks per instruction), v = v_hi + v_lo/16.
"""

import numpy as np
from contextlib import ExitStack

import concourse.bass as bass
import concourse.tile as tile
from concourse import bacc, mybir
from concourse.bass_utils import run_bass_kernel_spmd
from concourse.masks import make_identity

F32 = mybir.dt.float32
BF16 = mybir.dt.bfloat16
F8 = mybir.dt.float8e4

N, D, C, L = 8192, 256, 8, 7
S = N // C            # 1024 rows per core
KC = N // 128         # 64 k-chunks
MQ = S // 128         # 8 m-chunks per slab
G_AG = 2              # allgather split groups
A_DT = F8             # resident adjacency dtype ({0,1,2} exact in fp8e4)
MM1_F8 = True         # fp8 DoubleRow mm1 (u as hi + 16*lo fp8 pair)


def build_nc(n=N, d=D, c=C, n_layers=L, repeat=1, g_ag=G_AG, a_dt=A_DT,
             mm1_f8=MM1_F8, no_ag=False):
    s = n // c
    kc = n // 128
    mq = s // 128
    nmh = s // 512 if s >= 512 else 1   # moving halves of 512 (psum bank)
    mw = min(s, 512)                    # moving width
    dh_n = d // 128                     # d chunks (2 for d=256)
    G = g_ag
    sgk = (s // G) // 128               # k-chunks per (core, group)
    gm = mq // G                        # m-chunks per group
    gk = kc // G                        # k-chunks per group (all cores)
    assert d % 128 == 0 and s % (128 * G) == 0
    if mm1_f8:
        assert a_dt == F8 and sgk % 2 == 0
    u_dt = F8 if mm1_f8 else BF16
    ud_n = 2 if mm1_f8 else 1           # u planes (hi, scaled lo)
    dr = mybir.MatmulPerfMode.DoubleRow

    nc = bacc.Bacc("TRN2", target_bir_lowering=False, debug=False, num_devices=c)

    a_dram = nc.dram_tensor("a_slab", [n, s], F32, kind="ExternalInput")
    u0_dram = nc.dram_tensor("u0", [n, ud_n * d], u_dt, kind="ExternalInput")
    h0s_dram = nc.dram_tensor("h0_slab", [s, d], F32, kind="ExternalInput")
    dslab_dram = nc.dram_tensor("dinv_slab", [128, mq], F32, kind="ExternalInput")
    dbc_dram = nc.dram_tensor("dinv_bcast", [128, s], F32, kind="ExternalInput")
    wt_dram = nc.dram_tensor("wt", [n_layers, d, d], BF16, kind="ExternalInput")
    bias_dram = nc.dram_tensor("bias_t", [128, 2 * n_layers], F32, kind="ExternalInput")
    out_dram = nc.dram_tensor("out", [4, s, d], F32, kind="ExternalOutput")

    with ExitStack() as ctx:
        tc = ctx.enter_context(tile.TileContext(nc))
        dram = ctx.enter_context(tc.tile_pool(name="dram", bufs=1, space="DRAM"))
        res = ctx.enter_context(tc.tile_pool(name="res", bufs=1))
        stage = ctx.enter_context(tc.tile_pool(name="stage", bufs=2))
        up = ctx.enter_context(tc.tile_pool(name="up", bufs=2))
        wtp = ctx.enter_context(tc.tile_pool(name="wtp", bufs=2))
        work = ctx.enter_context(tc.tile_pool(name="work", bufs=2))
        slabp = ctx.enter_context(tc.tile_pool(name="slabp", bufs=2))
        pmm1 = ctx.enter_context(tc.tile_pool(name="pmm1", bufs=4, space="PSUM"))
        post = ctx.enter_context(tc.tile_pool(name="post", bufs=4, space="PSUM"))

        # ---- persistent DRAM scratch ----
        ag_ins = [
            dram.tile([s // G, ud_n * d], u_dt, name=f"ag_in{j}", tag=f"ag_in{j}",
                      bufs=2)
            for j in range(G)
        ]
        ag_outs = [
            [
                dram.tile(
                    [(n // G), ud_n * d], u_dt, name=f"ag_out{i}_{j}",
                    tag=f"ag_out{i}_{j}", addr_space="Shared",
                )
                for j in range(G)
            ]
            for i in range((n_layers - 1) * repeat)
        ]
        skip_dram = dram.tile([3, s, d], F32, name="skip_dram")

        # ---- persistent SBUF ----
        a_sb = res.tile([128, kc, s], a_dt, name="a_sb")
        dinv_sb = res.tile([128, mq], F32, name="dinv_sb")
        dinv_bc = res.tile([128, s], F32, name="dinv_bc")
        bias_sb = res.tile([128, 2 * n_layers], F32, name="bias_sb")
        ident = res.tile([128, 128], F32, name="ident")

        make_identity(nc, ident)
        nc.sync.dma_start(out=dinv_sb, in_=dslab_dram[:, :])
        nc.sync.dma_start(out=dinv_bc, in_=dbc_dram[:, :])
        nc.sync.dma_start(out=bias_sb, in_=bias_dram[:, :])

        # ---- startup: load A column-slab, cast to a_dt, fully resident ----
        for k in range(kc):
            st = stage.tile([128, s], F32, name="st", tag="stage")
            nc.sync.dma_start(out=st, in_=a_dram[k * 128 : (k + 1) * 128, :])
            nc.vector.tensor_copy(a_sb[:, k, :], st)

        relu = mybir.ActivationFunctionType.Relu
        skip_slot = {4: 2, 5: 1, 6: 0}  # up-layer l uses skip h_{...} slot

        # u0 grouped view: [ci, g, p, kk, d-planes] (per-ci DMAs at layer 0)
        u0_g = u0_dram[:, :].rearrange(
            "(ci g kk p) d2 -> ci g p kk d2", g=G, kk=sgk, p=128
        )

        for rep_l in range(n_layers * repeat):
            rep, l = divmod(rep_l, n_layers)
            # ---- Phase A: load U per gather group (flat t = ci*sgk+kk) ----
            u_hi = [
                up.tile([128, gk, d], u_dt, name=f"u_hi{j}", tag=f"u_hi{j}")
                for j in range(G)
            ]
            u_lo = [
                up.tile([128, gk, d], u_dt, name=f"u_lo{j}", tag=f"u_lo{j}")
                for j in range(G)
            ] if mm1_f8 else None
            for j in range(G):
                if l == 0:
                    for ci in range(c):
                        tsl = slice(ci * sgk, (ci + 1) * sgk)
                        nc.sync.dma_start(
                            out=u_hi[j][:, tsl, :], in_=u0_g[ci, j][:, :, 0:d]
                        )
                        if mm1_f8:
                            nc.sync.dma_start(
                                out=u_lo[j][:, tsl, :],
                                in_=u0_g[ci, j][:, :, d : 2 * d],
                            )
                else:
                    src = ag_outs[rep * (n_layers - 1) + l - 1][j]
                    sv = src.rearrange("(t p) d2 -> p t d2", p=128)
                    nc.sync.dma_start(out=u_hi[j], in_=sv[:, :, 0:d])
                    if mm1_f8:
                        nc.sync.dma_start(out=u_lo[j], in_=sv[:, :, d : 2 * d])

            # per-layer weight prefetch (bf16)
            wt_t = wtp.tile([128, dh_n, d], BF16, name="wt_t", tag="wt")
            nc.sync.dma_start(
                out=wt_t, in_=wt_dram[l].rearrange("(kc p) o -> p kc o", p=128)
            )

            # skip-connection preload for NEXT layer's input (scaled by dinv)
            nl = l + 1
            skip_sb = None
            if nl in skip_slot and nl < n_layers:
                skip_sb = slabp.tile([128, mq, d], F32, name="skip_sb", tag="skip")
                nc.sync.dma_start(
                    out=skip_sb,
                    in_=skip_dram[skip_slot[nl]].rearrange(
                        "(m p) d2 -> p m d2", p=128
                    ),
                )
                for m in range(mq):
                    nc.vector.tensor_scalar(
                        out=skip_sb[:, m, :],
                        in0=skip_sb[:, m, :],
                        scalar1=dinv_sb[:, m : m + 1],
                        scalar2=None,
                        op0=mybir.AluOpType.mult,
                    )

            # ---- Phase B: mm1  v.T[d, m] accumulate over k ----
            v_sb = [
                work.tile([128, s], BF16, name="v_sb", tag="vsb") for _ in range(dh_n)
            ]
            if not mm1_f8:
                psv = [
                    [pmm1.tile([128, mw], F32, name="psv", tag="pmm1")
                     for _ in range(nmh)]
                    for _ in range(dh_n)
                ]
                for mh in range(nmh):
                    msl = slice(mh * mw, (mh + 1) * mw)
                    for j in range(G):
                        for t in range(gk):
                            ci, kk = divmod(t, sgk)
                            k = ci * (s // 128) + j * sgk + kk
                            rhs = a_sb[:, k, msl]
                            for dh in range(dh_n):
                                nc.tensor.matmul(
                                    psv[dh][mh],
                                    u_hi[j][:, t, dh * 128 : (dh + 1) * 128],
                                    rhs,
                                    start=(j == 0 and t == 0),
                                    stop=(j == G - 1 and t == gk - 1),
                                )
                    for dh in range(dh_n):
                        nc.vector.tensor_copy(v_sb[dh][:, msl], psv[dh][mh])
            else:
                for mh in range(nmh):
                    msl = slice(mh * mw, (mh + 1) * mw)
                    ph = [pmm1.tile([128, mw], F32, name="ph", tag="pmm1")
                          for _ in range(dh_n)]
                    pl = [pmm1.tile([128, mw], F32, name="pl", tag="pmm1")
                          for _ in range(dh_n)]
                    for j in range(G):
                        for tp_i in range(gk // 2):
                            t0 = 2 * tp_i
                            ci, kk0 = divmod(t0, sgk)
                            k0 = ci * (s // 128) + j * sgk + kk0
                            rhs = a_sb[:, k0 : k0 + 2, msl]
                            st_ = (j == 0 and tp_i == 0)
                            sp_ = (j == G - 1 and tp_i == gk // 2 - 1)
                            for dh in range(dh_n):
                                dsl = slice(dh * 128, (dh + 1) * 128)
                                nc.tensor.matmul(
                                    ph[dh], u_hi[j][:, t0 : t0 + 2, dsl], rhs,
                                    start=st_, stop=sp_, perf_mode=dr,
                                )
                                nc.tensor.matmul(
                                    pl[dh], u_lo[j][:, t0 : t0 + 2, dsl], rhs,
                                    start=st_, stop=sp_, perf_mode=dr,
                                )
                    for dh in range(dh_n):
                        # v = v_hi + v_lo/16 (lo was stored as 16*residual)
                        vtmp = stage.tile([128, mw], BF16, name="vtmp", tag="vtmp")
                        nc.scalar.activation(
                            vtmp, pl[dh], mybir.ActivationFunctionType.Copy,
                            scale=0.0625,
                        )
                        nc.vector.tensor_add(v_sb[dh][:, msl], ph[dh], vtmp)

            # ---- Phase D/E per m-half: mm2, relu, transpose, u-prep ----
            is_out = l >= n_layers - 3  # layers 4,5,6 emit outputs 0,1,2
            save_skip = l <= 2
            h_nat = None
            if is_out or save_skip:
                h_nat = slabp.tile([128, mq, d], F32, name="h_nat", tag="hnat", bufs=1)
            if l == n_layers - 1:
                # reuse the (now idle) skip/us tags for the final-layer tiles
                h0s = slabp.tile([128, mq, d], F32, name="h0s", tag="skip")
                nc.sync.dma_start(
                    out=h0s, in_=h0s_dram[:, :].rearrange("(m p) d2 -> p m d2", p=128)
                )
                out3 = slabp.tile([128, mq, d], F32, name="out3", tag="us_hi", bufs=1)
            if l < n_layers - 1:
                us = slabp.tile(
                    [128, mq, ud_n * d], u_dt, name="us", tag="us_hi", bufs=1
                )

            hT = [work.tile([128, s], F32, name="hT", tag="hT") for _ in range(dh_n)]
            mq_h = mw // 128  # m-chunks per half
            for mh in range(nmh):
                msl = slice(mh * mw, (mh + 1) * mw)
                pso = [
                    post.tile([128, mw], F32, name="pso", tag="post")
                    for _ in range(dh_n)
                ]
                for dho in range(dh_n):
                    for kin in range(dh_n):
                        nc.tensor.matmul(
                            pso[dho],
                            wt_t[:, kin, dho * 128 : (dho + 1) * 128],
                            v_sb[kin][:, msl],
                            start=(kin == 0),
                            stop=(kin == dh_n - 1),
                        )
                for dho in range(dh_n):
                    nc.vector.tensor_mul(hT[dho][:, msl], pso[dho], dinv_bc[:, msl])
                    nc.scalar.activation(
                        hT[dho][:, msl],
                        hT[dho][:, msl],
                        relu,
                        bias=bias_sb[:, 2 * l + dho : 2 * l + dho + 1],
                    )
                for m in range(mh * mq_h, (mh + 1) * mq_h):
                    tp = post.tile([128, d], F32, name="tp", tag="post")
                    for dh in range(dh_n):
                        nc.tensor.transpose(
                            tp[:, dh * 128 : (dh + 1) * 128],
                            hT[dh][:, m * 128 : (m + 1) * 128],
                            ident,
                        )
                    if l < n_layers - 1:
                        ufp = stage.tile([128, d], F32, name="ufp", tag="ufp")
                        dv = dinv_sb[:, m : m + 1]
                        if skip_sb is not None:
                            nc.vector.scalar_tensor_tensor(
                                out=ufp,
                                in0=tp,
                                scalar=dv,
                                in1=skip_sb[:, m, :],
                                op0=mybir.AluOpType.mult,
                                op1=mybir.AluOpType.add,
                            )
                        else:
                            nc.vector.tensor_scalar(
                                out=ufp,
                                in0=tp,
                                scalar1=dv,
                                scalar2=None,
                                op0=mybir.AluOpType.mult,
                            )
                        nc.vector.tensor_copy(us[:, m, 0:d], ufp)
                        if mm1_f8:
                            ulo = stage.tile([128, d], F32, name="ulo", tag="ulo")
                            nc.vector.tensor_sub(ulo, ufp, us[:, m, 0:d])
                            nc.scalar.activation(
                                us[:, m, d : 2 * d], ulo,
                                mybir.ActivationFunctionType.Copy, scale=16.0,
                            )
                    if h_nat is not None:
                        nc.scalar.copy(h_nat[:, m, :], tp)
                    if l == n_layers - 1:
                        nc.vector.tensor_add(out3[:, m, :], tp, h0s[:, m, :])
                    # group complete -> stage + allgather immediately
                    if l < n_layers - 1 and (m + 1) % gm == 0:
                        j = m // gm
                        agi = ag_ins[j]
                        nc.sync.dma_start(
                            out=agi.rearrange("(mm p) d2 -> p mm d2", p=128),
                            in_=us[:, j * gm : (j + 1) * gm, :],
                        )
                        if not no_ag:
                            nc.gpsimd.collective_compute(
                                "AllGather",
                                mybir.AluOpType.bypass,
                                replica_groups=[list(range(c))],
                                ins=[agi.opt()],
                                outs=[ag_outs[rep * (n_layers - 1) + l][j].opt()],
                            )
                        else:
                            # timing-only mode: fake the gather with a local
                            # DMA of the slab into own block of the output
                            nc.sync.dma_start(
                                out=ag_outs[rep * (n_layers - 1) + l][j][
                                    0 : s // G, :
                                ],
                                in_=agi[:, :],
                            )

            # ---- Phase F: DMAs out ----
            if save_skip:
                nc.sync.dma_start(
                    out=skip_dram[l].rearrange("(m p) d2 -> p m d2", p=128),
                    in_=h_nat,
                )
            if is_out:
                nc.sync.dma_start(
                    out=out_dram[l - (n_layers - 3)].rearrange(
                        "(m p) d2 -> p m d2", p=128
                    ),
                    in_=h_nat,
                )
            if l == n_layers - 1:
                nc.sync.dma_start(
                    out=out_dram[3].rearrange("(m p) d2 -> p m d2", p=128), in_=out3
                )

    nc.compile()
    return nc


try:
    import ml_dtypes

    ml_bf16 = ml_dtypes.bfloat16
    ml_f8 = ml_dtypes.float8_e4m3fn
except ImportError:  # pragma: no cover
    import jax.numpy as jnp

    ml_bf16 = jnp.bfloat16
    ml_f8 = jnp.float8_e4m3fn


def prep_inputs(g, h, W_down, b_down, W_bottom, b_bottom, W_up, b_up, c=C,
                mm1_f8=MM1_F8):
    """Host-side sharding + layout prep. Returns per-core input maps."""
    n = g.shape[0]
    s = n // c
    d = h.shape[1]
    g = np.asarray(g, np.float32)
    h = np.asarray(h, np.float32)
    deg = g.sum(axis=1) + 1.0
    dinv = (1.0 / np.sqrt(deg)).astype(np.float32)

    u0 = (h * dinv[:, None]).astype(np.float32)
    if mm1_f8:
        u0_hi = u0.astype(ml_f8)
        u0_lo = ((u0 - u0_hi.astype(np.float32)) * 16.0).astype(ml_f8)
        u0_packed = np.concatenate([np.asarray(u0_hi), np.asarray(u0_lo)], axis=1)
    else:
        u0_packed = np.asarray(u0.astype(ml_bf16))  # [n, d] bf16

    Ws = [W_down[0], W_down[1], W_down[2], W_bottom, W_up[0], W_up[1], W_up[2]]
    bs = [b_down[0], b_down[1], b_down[2], b_bottom, b_up[0], b_up[1], b_up[2]]
    wt = np.stack(
        [np.ascontiguousarray(np.asarray(W, np.float32).T).astype(ml_bf16) for W in Ws]
    )
    nl = len(Ws)
    bias_t = np.zeros((128, 2 * nl), np.float32)
    for li, b in enumerate(bs):
        b = np.asarray(b, np.float32)
        for dh in range(d // 128):
            bias_t[:, 2 * li + dh] = b[dh * 128 : (dh + 1) * 128]

    in_maps = []
    for ci in range(c):
        sl = slice(ci * s, (ci + 1) * s)
        a_slab = np.ascontiguousarray(g[:, sl])
        idx = np.arange(s)
        a_slab[ci * s + idx, idx] += 1.0  # fold self-loops into the slab
        dinv_slab = dinv[sl].reshape(s // 128, 128).T.copy()  # [128, mq]
        dinv_bcast = np.broadcast_to(dinv[sl][None, :], (128, s)).copy()
        in_maps.append(
            dict(
                a_slab=a_slab,
                u0=u0_packed,
                h0_slab=np.ascontiguousarray(h[sl]),
                dinv_slab=dinv_slab,
                dinv_bcast=dinv_bcast,
                wt=wt,
                bias_t=bias_t,
            )
        )
    return in_maps


_NC_CACHE = {}


def kernel(g, h, W_down, b_down, W_bottom, b_bottom, W_up, b_up):
    key = "full"
    if key not in _NC_CACHE:
        _NC_CACHE[key] = build_nc()
    nc = _NC_CACHE[key]
    in_maps = prep_inputs(g, h, W_down, b_down, W_bottom, b_bottom, W_up, b_up)
    res = run_bass_kernel_spmd(nc, in_maps, list(range(C)))
    outs = [np.asarray(r["out"]).reshape(4, S, D) for r in res.results]
    full = np.concatenate(outs, axis=1)  # [4, N, D]
    return full.astype(np.float32)


if __name__ == "__main__":
    import reference

    inputs = reference.setup_inputs()
    inputs = {k: np.asarray(v) for k, v in inputs.items()}
    out = kernel(**inputs)
    exp = np.asarray(reference.reference(**reference.setup_inputs()))
    err = np.abs(out - exp).max() / (np.abs(exp).max() + 1e-30)
    rel = np.linalg.norm(out - exp) / (np.linalg.norm(exp) + 1e-30)
    print("max-scaled err:", err, "rel l2:", rel)


# revision 14
# speedup vs baseline: 2.5040x; 1.3980x over previous
"""GraphUnetNoPool (7-layer GCN U-net, no pooling) on 8 trn2 NeuronCores.

Math: gn = D^-1/2 (g+I) D^-1/2;  layer: h' = relu(gn @ h @ W.T + b)
Rewrite: u = dinv*h;  v = (g+I) @ u;  h' = relu((dinv*v) @ W.T + b)
  =>  per-core row-slab m:  v.T[d, m] = sum_k u[k, d] * A[k, m]  (A symmetric:
      column slab of A == transposed row slab, so lhsT = u natural layout and
      rhs = A[:, slab] streams naturally from DRAM rows).
Sharding: 1D row-parallel. Core c owns rows [c*S, (c+1)*S).

v2: mm1 in single bf16 (u rounded to bf16; A exact in fp8), A fully resident
in SBUF as fp8, mm2/weights in bf16, per-layer AllGather of u split into G
groups, each issued as soon as its row block is ready so the collective
overlaps the next layer's mm1 (which consumes gathered groups in order).
v3 (mm1_f8): u as fp8 hi + 16*lo pair, mm1 via DoubleRow fp8 matmuls (2
k-chunks per instruction), v = v_hi + v_lo/16.
"""

import numpy as np
from contextlib import ExitStack

import concourse.bass as bass
import concourse.tile as tile
from concourse import bacc, mybir
from concourse.bass_utils import run_bass_kernel_spmd
from concourse.masks import make_identity

F32 = mybir.dt.float32
BF16 = mybir.dt.bfloat16
F8 = mybir.dt.float8e4

N, D, C, L = 8192, 256, 8, 7
S = N // C            # 1024 rows per core
KC = N // 128         # 64 k-chunks
MQ = S // 128         # 8 m-chunks per slab
G_AG = 2              # allgather split groups
A_DT = F8             # resident adjacency dtype ({0,1,2} exact in fp8e4)
MM1_F8 = True         # fp8 DoubleRow mm1 (u as hi + 16*lo fp8 pair)
F8_SINGLE = True      # single scaled-e4m3 plane (u*32); halves wire + mm1
U_SCALE = 32.0


def build_nc(n=N, d=D, c=C, n_layers=L, repeat=1, g_ag=G_AG, a_dt=A_DT,
             mm1_f8=MM1_F8, no_ag=False, ag_tiny=False, f8_single=F8_SINGLE):
    s = n // c
    kc = n // 128
    mq = s // 128
    nmh = s // 512 if s >= 512 else 1   # moving halves of 512 (psum bank)
    mw = min(s, 512)                    # moving width
    dh_n = d // 128                     # d chunks (2 for d=256)
    G = g_ag
    sgk = (s // G) // 128               # k-chunks per (core, group)
    gm = mq // G                        # m-chunks per group
    gk = kc // G                        # k-chunks per group (all cores)
    assert d % 128 == 0 and s % (128 * G) == 0
    if mm1_f8:
        assert a_dt == F8 and sgk % 2 == 0
    u_dt = F8 if mm1_f8 else BF16
    use_lo = mm1_f8 and not f8_single
    ud_n = 2 if use_lo else 1           # u planes (hi [, scaled lo])
    dr = mybir.MatmulPerfMode.DoubleRow

    nc = bacc.Bacc("TRN2", target_bir_lowering=False, debug=False, num_devices=c)

    a_dram = nc.dram_tensor("a_slab", [n, s], F32, kind="ExternalInput")
    u0_dram = nc.dram_tensor("u0", [n, ud_n * d], u_dt, kind="ExternalInput")
    h0s_dram = nc.dram_tensor("h0_slab", [s, d], F32, kind="ExternalInput")
    dslab_dram = nc.dram_tensor("dinv_slab", [128, mq], F32, kind="ExternalInput")
    dbc_dram = nc.dram_tensor("dinv_bcast", [128, s], F32, kind="ExternalInput")
    wt_dram = nc.dram_tensor("wt", [n_layers, d, d], BF16, kind="ExternalInput")
    bias_dram = nc.dram_tensor("bias_t", [128, 2 * n_layers], F32, kind="ExternalInput")
    out_dram = nc.dram_tensor("out", [4, s, d], F32, kind="ExternalOutput")

    with ExitStack() as ctx:
        tc = ctx.enter_context(tile.TileContext(nc))
        dram = ctx.enter_context(tc.tile_pool(name="dram", bufs=1, space="DRAM"))
        res = ctx.enter_context(tc.tile_pool(name="res", bufs=1))
        stage = ctx.enter_context(tc.tile_pool(name="stage", bufs=2))
        up = ctx.enter_context(tc.tile_pool(name="up", bufs=2))
        wtp = ctx.enter_context(tc.tile_pool(name="wtp", bufs=2))
        work = ctx.enter_context(tc.tile_pool(name="work", bufs=2))
        slabp = ctx.enter_context(tc.tile_pool(name="slabp", bufs=2))
        pmm1 = ctx.enter_context(tc.tile_pool(name="pmm1", bufs=4, space="PSUM"))
        post = ctx.enter_context(tc.tile_pool(name="post", bufs=4, space="PSUM"))

        # ---- persistent DRAM scratch ----
        ag_ins = [
            dram.tile([s // G, ud_n * d], u_dt, name=f"ag_in{j}", tag=f"ag_in{j}",
                      bufs=2)
            for j in range(G)
        ]
        if ag_tiny:
            agt_ins = [
                dram.tile([8, ud_n * d], u_dt, name=f"agt_in{j}", tag=f"agt_in{j}",
                          bufs=2)
                for j in range(G)
            ]
            agt_outs = [
                [
                    dram.tile(
                        [8 * c, ud_n * d], u_dt, name=f"agt_out{i}_{j}",
                        tag=f"agt_out{i}_{j}", addr_space="Shared",
                    )
                    for j in range(G)
                ]
                for i in range((n_layers - 1) * repeat)
            ]
        ag_outs = [
            [
                dram.tile(
                    [(n // G), ud_n * d], u_dt, name=f"ag_out{i}_{j}",
                    tag=f"ag_out{i}_{j}", addr_space="Shared",
                )
                for j in range(G)
            ]
            for i in range((n_layers - 1) * repeat)
        ]
        skip_dram = dram.tile([3, s, d], F32, name="skip_dram")

        # ---- persistent SBUF ----
        a_sb = res.tile([128, kc, s], a_dt, name="a_sb")
        dinv_sb = res.tile([128, mq], F32, name="dinv_sb")
        dinv_bc = res.tile([128, s], F32, name="dinv_bc")
        bias_sb = res.tile([128, 2 * n_layers], F32, name="bias_sb")
        ident = res.tile([128, 128], F32, name="ident")

        make_identity(nc, ident)
        nc.sync.dma_start(out=dinv_sb, in_=dslab_dram[:, :])
        nc.sync.dma_start(out=dinv_bc, in_=dbc_dram[:, :])
        nc.sync.dma_start(out=bias_sb, in_=bias_dram[:, :])

        # ---- startup: load A column-slab, cast to a_dt, fully resident ----
        for k in range(kc):
            st = stage.tile([128, s], F32, name="st", tag="stage")
            nc.sync.dma_start(out=st, in_=a_dram[k * 128 : (k + 1) * 128, :])
            nc.vector.tensor_copy(a_sb[:, k, :], st)

        relu = mybir.ActivationFunctionType.Relu
        skip_slot = {4: 2, 5: 1, 6: 0}  # up-layer l uses skip h_{...} slot

        # u0 grouped view: [ci, g, p, kk, d-planes] (per-ci DMAs at layer 0)
        u0_g = u0_dram[:, :].rearrange(
            "(ci g kk p) d2 -> ci g p kk d2", g=G, kk=sgk, p=128
        )

        for rep_l in range(n_layers * repeat):
            rep, l = divmod(rep_l, n_layers)
            # ---- Phase A: load U per gather group (flat t = ci*sgk+kk) ----
            u_hi = [
                up.tile([128, gk, d], u_dt, name=f"u_hi{j}", tag=f"u_hi{j}")
                for j in range(G)
            ]
            u_lo = [
                up.tile([128, gk, d], u_dt, name=f"u_lo{j}", tag=f"u_lo{j}")
                for j in range(G)
            ] if use_lo else None
            for j in range(G):
                if l == 0:
                    for ci in range(c):
                        tsl = slice(ci * sgk, (ci + 1) * sgk)
                        nc.sync.dma_start(
                            out=u_hi[j][:, tsl, :], in_=u0_g[ci, j][:, :, 0:d]
                        )
                        if use_lo:
                            nc.sync.dma_start(
                                out=u_lo[j][:, tsl, :],
                                in_=u0_g[ci, j][:, :, d : 2 * d],
                            )
                else:
                    src = ag_outs[rep * (n_layers - 1) + l - 1][j]
                    sv = src.rearrange("(t p) d2 -> p t d2", p=128)
                    nc.sync.dma_start(out=u_hi[j], in_=sv[:, :, 0:d])
                    if use_lo:
                        nc.sync.dma_start(out=u_lo[j], in_=sv[:, :, d : 2 * d])

            # per-layer weight prefetch (bf16)
            wt_t = wtp.tile([128, dh_n, d], BF16, name="wt_t", tag="wt")
            nc.sync.dma_start(
                out=wt_t, in_=wt_dram[l].rearrange("(kc p) o -> p kc o", p=128)
            )

            # skip-connection preload for NEXT layer's input (scaled by dinv)
            nl = l + 1
            skip_sb = None
            if nl in skip_slot and nl < n_layers:
                skip_sb = slabp.tile([128, mq, d], F32, name="skip_sb", tag="skip")
                nc.sync.dma_start(
                    out=skip_sb,
                    in_=skip_dram[skip_slot[nl]].rearrange(
                        "(m p) d2 -> p m d2", p=128
                    ),
                )
                for m in range(mq):
                    nc.vector.tensor_scalar(
                        out=skip_sb[:, m, :],
                        in0=skip_sb[:, m, :],
                        scalar1=dinv_sb[:, m : m + 1],
                        scalar2=None,
                        op0=mybir.AluOpType.mult,
                    )

            # ---- Phase B: mm1  v.T[d, m] accumulate over k ----
            v_sb = [
                work.tile([128, s], BF16, name="v_sb", tag="vsb") for _ in range(dh_n)
            ]
            if not mm1_f8:
                psv = [
                    [pmm1.tile([128, mw], F32, name="psv", tag="pmm1")
                     for _ in range(nmh)]
                    for _ in range(dh_n)
                ]
                for mh in range(nmh):
                    msl = slice(mh * mw, (mh + 1) * mw)
                    for j in range(G):
                        for t in range(gk):
                            ci, kk = divmod(t, sgk)
                            k = ci * (s // 128) + j * sgk + kk
                            rhs = a_sb[:, k, msl]
                            for dh in range(dh_n):
                                nc.tensor.matmul(
                                    psv[dh][mh],
                                    u_hi[j][:, t, dh * 128 : (dh + 1) * 128],
                                    rhs,
                                    start=(j == 0 and t == 0),
                                    stop=(j == G - 1 and t == gk - 1),
                                )
                    for dh in range(dh_n):
                        nc.vector.tensor_copy(v_sb[dh][:, msl], psv[dh][mh])
            else:
                for mh in range(nmh):
                    msl = slice(mh * mw, (mh + 1) * mw)
                    ph = [pmm1.tile([128, mw], F32, name="ph", tag="pmm1")
                          for _ in range(dh_n)]
                    pl = [pmm1.tile([128, mw], F32, name="pl", tag="pmm1")
                          for _ in range(dh_n)] if use_lo else None
                    for j in range(G):
                        for tp_i in range(gk // 2):
                            t0 = 2 * tp_i
                            ci, kk0 = divmod(t0, sgk)
                            k0 = ci * (s // 128) + j * sgk + kk0
                            rhs = a_sb[:, k0 : k0 + 2, msl]
                            st_ = (j == 0 and tp_i == 0)
                            sp_ = (j == G - 1 and tp_i == gk // 2 - 1)
                            for dh in range(dh_n):
                                dsl = slice(dh * 128, (dh + 1) * 128)
                                nc.tensor.matmul(
                                    ph[dh], u_hi[j][:, t0 : t0 + 2, dsl], rhs,
                                    start=st_, stop=sp_, perf_mode=dr,
                                )
                                if use_lo:
                                    nc.tensor.matmul(
                                        pl[dh], u_lo[j][:, t0 : t0 + 2, dsl], rhs,
                                        start=st_, stop=sp_, perf_mode=dr,
                                    )
                    for dh in range(dh_n):
                        if use_lo:
                            # v = v_hi + v_lo/16 (lo was stored as 16*residual)
                            vtmp = stage.tile([128, mw], BF16, name="vtmp", tag="vtmp")
                            nc.scalar.activation(
                                vtmp, pl[dh], mybir.ActivationFunctionType.Copy,
                                scale=0.0625,
                            )
                            nc.vector.tensor_add(v_sb[dh][:, msl], ph[dh], vtmp)
                        else:
                            nc.vector.tensor_copy(v_sb[dh][:, msl], ph[dh])

            # ---- Phase D/E per m-half: mm2, relu, transpose, u-prep ----
            is_out = l >= n_layers - 3  # layers 4,5,6 emit outputs 0,1,2
            save_skip = l <= 2
            h_nat = None
            if is_out or save_skip:
                h_nat = slabp.tile([128, mq, d], F32, name="h_nat", tag="hnat", bufs=1)
            if l == n_layers - 1:
                # reuse the (now idle) skip/us tags for the final-layer tiles
                h0s = slabp.tile([128, mq, d], F32, name="h0s", tag="skip")
                nc.sync.dma_start(
                    out=h0s, in_=h0s_dram[:, :].rearrange("(m p) d2 -> p m d2", p=128)
                )
                out3 = slabp.tile([128, mq, d], F32, name="out3", tag="us_hi", bufs=1)
            if l < n_layers - 1:
                us = slabp.tile(
                    [128, mq, ud_n * d], u_dt, name="us", tag="us_hi", bufs=1
                )

            hT = [work.tile([128, s], F32, name="hT", tag="hT") for _ in range(dh_n)]
            mq_h = mw // 128  # m-chunks per half
            for mh in range(nmh):
                msl = slice(mh * mw, (mh + 1) * mw)
                pso = [
                    post.tile([128, mw], F32, name="pso", tag="post")
                    for _ in range(dh_n)
                ]
                for dho in range(dh_n):
                    for kin in range(dh_n):
                        nc.tensor.matmul(
                            pso[dho],
                            wt_t[:, kin, dho * 128 : (dho + 1) * 128],
                            v_sb[kin][:, msl],
                            start=(kin == 0),
                            stop=(kin == dh_n - 1),
                        )
                for dho in range(dh_n):
                    nc.vector.tensor_mul(hT[dho][:, msl], pso[dho], dinv_bc[:, msl])
                    nc.scalar.activation(
                        hT[dho][:, msl],
                        hT[dho][:, msl],
                        relu,
                        bias=bias_sb[:, 2 * l + dho : 2 * l + dho + 1],
                    )
                for m in range(mh * mq_h, (mh + 1) * mq_h):
                    tp = post.tile([128, d], F32, name="tp", tag="post")
                    for dh in range(dh_n):
                        nc.tensor.transpose(
                            tp[:, dh * 128 : (dh + 1) * 128],
                            hT[dh][:, m * 128 : (m + 1) * 128],
                            ident,
                        )
                    if l < n_layers - 1:
                        ufp = stage.tile([128, d], F32, name="ufp", tag="ufp")
                        dv = dinv_sb[:, m : m + 1]
                        if skip_sb is not None:
                            nc.vector.scalar_tensor_tensor(
                                out=ufp,
                                in0=tp,
                                scalar=dv,
                                in1=skip_sb[:, m, :],
                                op0=mybir.AluOpType.mult,
                                op1=mybir.AluOpType.add,
                            )
                        else:
                            nc.vector.tensor_scalar(
                                out=ufp,
                                in0=tp,
                                scalar1=dv,
                                scalar2=None,
                                op0=mybir.AluOpType.mult,
                            )
                        if mm1_f8 and f8_single:
                            nc.vector.tensor_scalar(
                                out=us[:, m, 0:d], in0=ufp, scalar1=U_SCALE,
                                scalar2=None, op0=mybir.AluOpType.mult,
                            )
                        else:
                            nc.vector.tensor_copy(us[:, m, 0:d], ufp)
                        if use_lo:
                            ulo = stage.tile([128, d], F32, name="ulo", tag="ulo")
                            nc.vector.tensor_sub(ulo, ufp, us[:, m, 0:d])
                            nc.scalar.activation(
                                us[:, m, d : 2 * d], ulo,
                                mybir.ActivationFunctionType.Copy, scale=16.0,
                            )
                    if h_nat is not None:
                        nc.scalar.copy(h_nat[:, m, :], tp)
                    if l == n_layers - 1:
                        nc.vector.tensor_add(out3[:, m, :], tp, h0s[:, m, :])
                    # group complete -> stage + allgather immediately
                    if l < n_layers - 1 and (m + 1) % gm == 0:
                        j = m // gm
                        agi = ag_ins[j]
                        nc.sync.dma_start(
                            out=agi.rearrange("(mm p) d2 -> p mm d2", p=128),
                            in_=us[:, j * gm : (j + 1) * gm, :],
                        )
                        if ag_tiny:
                            # timing probe: same collective cadence, 4KB payload
                            agti = agt_ins[j]
                            nc.sync.dma_start(
                                out=agti[:, :], in_=us[0:8, j * gm, :]
                            )
                            nc.gpsimd.collective_compute(
                                "AllGather",
                                mybir.AluOpType.bypass,
                                replica_groups=[list(range(c))],
                                ins=[agti.opt()],
                                outs=[agt_outs[rep * (n_layers - 1) + l][j].opt()],
                            )
                        elif not no_ag:
                            nc.gpsimd.collective_compute(
                                "AllGather",
                                mybir.AluOpType.bypass,
                                replica_groups=[list(range(c))],
                                ins=[agi.opt()],
                                outs=[ag_outs[rep * (n_layers - 1) + l][j].opt()],
                            )
                        else:
                            # timing-only mode: fake the gather with a local
                            # DMA of the slab into own block of the output
                            nc.sync.dma_start(
                                out=ag_outs[rep * (n_layers - 1) + l][j][
                                    0 : s // G, :
                                ],
                                in_=agi[:, :],
                            )

            # ---- Phase F: DMAs out ----
            if save_skip:
                nc.sync.dma_start(
                    out=skip_dram[l].rearrange("(m p) d2 -> p m d2", p=128),
                    in_=h_nat,
                )
            if is_out:
                nc.sync.dma_start(
                    out=out_dram[l - (n_layers - 3)].rearrange(
                        "(m p) d2 -> p m d2", p=128
                    ),
                    in_=h_nat,
                )
            if l == n_layers - 1:
                nc.sync.dma_start(
                    out=out_dram[3].rearrange("(m p) d2 -> p m d2", p=128), in_=out3
                )

    nc.compile()
    return nc


try:
    import ml_dtypes

    ml_bf16 = ml_dtypes.bfloat16
    ml_f8 = ml_dtypes.float8_e4m3fn
except ImportError:  # pragma: no cover
    import jax.numpy as jnp

    ml_bf16 = jnp.bfloat16
    ml_f8 = jnp.float8_e4m3fn


def prep_inputs(g, h, W_down, b_down, W_bottom, b_bottom, W_up, b_up, c=C,
                mm1_f8=MM1_F8, f8_single=F8_SINGLE):
    """Host-side sharding + layout prep. Returns per-core input maps."""
    n = g.shape[0]
    s = n // c
    d = h.shape[1]
    g = np.asarray(g, np.float32)
    h = np.asarray(h, np.float32)
    deg = g.sum(axis=1) + 1.0
    dinv = (1.0 / np.sqrt(deg)).astype(np.float32)

    u0 = (h * dinv[:, None]).astype(np.float32)
    if mm1_f8 and f8_single:
        u0_packed = np.asarray((u0 * 32.0).astype(ml_f8))  # [n, d] scaled fp8
    elif mm1_f8:
        u0_hi = u0.astype(ml_f8)
        u0_lo = ((u0 - u0_hi.astype(np.float32)) * 16.0).astype(ml_f8)
        u0_packed = np.concatenate([np.asarray(u0_hi), np.asarray(u0_lo)], axis=1)
    else:
        u0_packed = np.asarray(u0.astype(ml_bf16))  # [n, d] bf16

    Ws = [W_down[0], W_down[1], W_down[2], W_bottom, W_up[0], W_up[1], W_up[2]]
    bs = [b_down[0], b_down[1], b_down[2], b_bottom, b_up[0], b_up[1], b_up[2]]
    wt = np.stack(
        [np.ascontiguousarray(np.asarray(W, np.float32).T).astype(ml_bf16) for W in Ws]
    )
    nl = len(Ws)
    bias_t = np.zeros((128, 2 * nl), np.float32)
    for li, b in enumerate(bs):
        b = np.asarray(b, np.float32)
        for dh in range(d // 128):
            bias_t[:, 2 * li + dh] = b[dh * 128 : (dh + 1) * 128]

    in_maps = []
    for ci in range(c):
        sl = slice(ci * s, (ci + 1) * s)
        a_slab = np.ascontiguousarray(g[:, sl])
        idx = np.arange(s)
        a_slab[ci * s + idx, idx] += 1.0  # fold self-loops into the slab
        dinv_slab = dinv[sl].reshape(s // 128, 128).T.copy()  # [128, mq]
        dsc = dinv / 32.0 if (mm1_f8 and f8_single) else dinv
        dinv_bcast = np.broadcast_to(dsc[sl][None, :], (128, s)).copy()
        in_maps.append(
            dict(
                a_slab=a_slab,
                u0=u0_packed,
                h0_slab=np.ascontiguousarray(h[sl]),
                dinv_slab=dinv_slab,
                dinv_bcast=dinv_bcast,
                wt=wt,
                bias_t=bias_t,
            )
        )
    return in_maps


_NC_CACHE = {}


def kernel(g, h, W_down, b_down, W_bottom, b_bottom, W_up, b_up):
    key = "full"
    if key not in _NC_CACHE:
        _NC_CACHE[key] = build_nc()
    nc = _NC_CACHE[key]
    in_maps = prep_inputs(g, h, W_down, b_down, W_bottom, b_bottom, W_up, b_up)
    res = run_bass_kernel_spmd(nc, in_maps, list(range(C)))
    outs = [np.asarray(r["out"]).reshape(4, S, D) for r in res.results]
    full = np.concatenate(outs, axis=1)  # [4, N, D]
    return full.astype(np.float32)


if __name__ == "__main__":
    import reference

    inputs = reference.setup_inputs()
    inputs = {k: np.asarray(v) for k, v in inputs.items()}
    out = kernel(**inputs)
    exp = np.asarray(reference.reference(**reference.setup_inputs()))
    err = np.abs(out - exp).max() / (np.abs(exp).max() + 1e-30)
    rel = np.linalg.norm(out - exp) / (np.linalg.norm(exp) + 1e-30)
    print("max-scaled err:", err, "rel l2:", rel)


# revision 15
# speedup vs baseline: 2.5333x; 1.0117x over previous
"""GraphUnetNoPool (7-layer GCN U-net, no pooling) on 8 trn2 NeuronCores.

Math: gn = D^-1/2 (g+I) D^-1/2;  layer: h' = relu(gn @ h @ W.T + b)
Rewrite: u = dinv*h;  v = (g+I) @ u;  h' = relu((dinv*v) @ W.T + b)
  =>  per-core row-slab m:  v.T[d, m] = sum_k u[k, d] * A[k, m]  (A symmetric:
      column slab of A == transposed row slab, so lhsT = u natural layout and
      rhs = A[:, slab] streams naturally from DRAM rows).
Sharding: 1D row-parallel. Core c owns rows [c*S, (c+1)*S).

v2: mm1 in single bf16 (u rounded to bf16; A exact in fp8), A fully resident
in SBUF as fp8, mm2/weights in bf16, per-layer AllGather of u split into G
groups, each issued as soon as its row block is ready so the collective
overlaps the next layer's mm1 (which consumes gathered groups in order).
v3 (mm1_f8): u as fp8 hi + 16*lo pair, mm1 via DoubleRow fp8 matmuls (2
k-chunks per instruction), v = v_hi + v_lo/16.
"""

import numpy as np
from contextlib import ExitStack

import concourse.bass as bass
import concourse.tile as tile
from concourse import bacc, mybir
from concourse.bass_utils import run_bass_kernel_spmd
from concourse.masks import make_identity

F32 = mybir.dt.float32
BF16 = mybir.dt.bfloat16
F8 = mybir.dt.float8e4

N, D, C, L = 8192, 256, 8, 7
S = N // C            # 1024 rows per core
KC = N // 128         # 64 k-chunks
MQ = S // 128         # 8 m-chunks per slab
G_AG = 2              # allgather split groups
A_DT = F8             # resident adjacency dtype ({0,1,2} exact in fp8e4)
MM1_F8 = True         # fp8 DoubleRow mm1 (u as hi + 16*lo fp8 pair)
F8_SINGLE = True      # single scaled-e4m3 plane (u*32); halves wire + mm1
U_SCALE = 32.0
K_OUTER = False       # mm1 k-outer: reuse each stationary for both m-halves


def build_nc(n=N, d=D, c=C, n_layers=L, repeat=1, g_ag=G_AG, a_dt=A_DT,
             mm1_f8=MM1_F8, no_ag=False, ag_tiny=False, f8_single=F8_SINGLE,
             k_outer=K_OUTER):
    s = n // c
    kc = n // 128
    mq = s // 128
    nmh = s // 512 if s >= 512 else 1   # moving halves of 512 (psum bank)
    mw = min(s, 512)                    # moving width
    dh_n = d // 128                     # d chunks (2 for d=256)
    G = g_ag
    sgk = (s // G) // 128               # k-chunks per (core, group)
    gm = mq // G                        # m-chunks per group
    gk = kc // G                        # k-chunks per group (all cores)
    assert d % 128 == 0 and s % (128 * G) == 0
    if mm1_f8:
        assert a_dt == F8 and sgk % 2 == 0
    u_dt = F8 if mm1_f8 else BF16
    use_lo = mm1_f8 and not f8_single
    ud_n = 2 if use_lo else 1           # u planes (hi [, scaled lo])
    dr = mybir.MatmulPerfMode.DoubleRow

    nc = bacc.Bacc("TRN2", target_bir_lowering=False, debug=False, num_devices=c)

    a_dram = nc.dram_tensor("a_slab", [n, s], F32, kind="ExternalInput")
    u0_dram = nc.dram_tensor("u0", [n, ud_n * d], u_dt, kind="ExternalInput")
    h0s_dram = nc.dram_tensor("h0_slab", [s, d], F32, kind="ExternalInput")
    dslab_dram = nc.dram_tensor("dinv_slab", [128, mq], F32, kind="ExternalInput")
    dbc_dram = nc.dram_tensor("dinv_bcast", [128, s], F32, kind="ExternalInput")
    wt_dram = nc.dram_tensor("wt", [n_layers, d, d], BF16, kind="ExternalInput")
    bias_dram = nc.dram_tensor("bias_t", [128, 2 * n_layers], F32, kind="ExternalInput")
    out_dram = nc.dram_tensor("out", [4, s, d], F32, kind="ExternalOutput")

    with ExitStack() as ctx:
        tc = ctx.enter_context(tile.TileContext(nc))
        dram = ctx.enter_context(tc.tile_pool(name="dram", bufs=1, space="DRAM"))
        res = ctx.enter_context(tc.tile_pool(name="res", bufs=1))
        stage = ctx.enter_context(tc.tile_pool(name="stage", bufs=2))
        up = ctx.enter_context(tc.tile_pool(name="up", bufs=2))
        wtp = ctx.enter_context(tc.tile_pool(name="wtp", bufs=2))
        work = ctx.enter_context(tc.tile_pool(name="work", bufs=2))
        slabp = ctx.enter_context(tc.tile_pool(name="slabp", bufs=2))
        pmm1 = ctx.enter_context(tc.tile_pool(name="pmm1", bufs=4, space="PSUM"))
        post = ctx.enter_context(tc.tile_pool(name="post", bufs=4, space="PSUM"))

        # ---- persistent DRAM scratch ----
        ag_ins = [
            dram.tile([s // G, ud_n * d], u_dt, name=f"ag_in{j}", tag=f"ag_in{j}",
                      bufs=2)
            for j in range(G)
        ]
        if ag_tiny:
            agt_ins = [
                dram.tile([8, ud_n * d], u_dt, name=f"agt_in{j}", tag=f"agt_in{j}",
                          bufs=2)
                for j in range(G)
            ]
            agt_outs = [
                [
                    dram.tile(
                        [8 * c, ud_n * d], u_dt, name=f"agt_out{i}_{j}",
                        tag=f"agt_out{i}_{j}", addr_space="Shared",
                    )
                    for j in range(G)
                ]
                for i in range((n_layers - 1) * repeat)
            ]
        ag_outs = [
            [
                dram.tile(
                    [(n // G), ud_n * d], u_dt, name=f"ag_out{i}_{j}",
                    tag=f"ag_out{i}_{j}", addr_space="Shared",
                )
                for j in range(G)
            ]
            for i in range((n_layers - 1) * repeat)
        ]
        skip_dram = dram.tile([3, s, d], F32, name="skip_dram")

        # ---- persistent SBUF ----
        a_sb = res.tile([128, kc, s], a_dt, name="a_sb")
        dinv_sb = res.tile([128, mq], F32, name="dinv_sb")
        dinv_bc = res.tile([128, s], F32, name="dinv_bc")
        bias_sb = res.tile([128, 2 * n_layers], F32, name="bias_sb")
        ident = res.tile([128, 128], F32, name="ident")

        make_identity(nc, ident)
        nc.sync.dma_start(out=dinv_sb, in_=dslab_dram[:, :])
        nc.sync.dma_start(out=dinv_bc, in_=dbc_dram[:, :])
        nc.sync.dma_start(out=bias_sb, in_=bias_dram[:, :])

        # ---- startup: load A column-slab, cast to a_dt, fully resident ----
        for k in range(kc):
            st = stage.tile([128, s], F32, name="st", tag="stage")
            nc.sync.dma_start(out=st, in_=a_dram[k * 128 : (k + 1) * 128, :])
            nc.vector.tensor_copy(a_sb[:, k, :], st)

        relu = mybir.ActivationFunctionType.Relu
        skip_slot = {4: 2, 5: 1, 6: 0}  # up-layer l uses skip h_{...} slot

        # u0 grouped view: [ci, g, p, kk, d-planes] (per-ci DMAs at layer 0)
        u0_g = u0_dram[:, :].rearrange(
            "(ci g kk p) d2 -> ci g p kk d2", g=G, kk=sgk, p=128
        )

        for rep_l in range(n_layers * repeat):
            rep, l = divmod(rep_l, n_layers)
            # ---- Phase A: load U per gather group (flat t = ci*sgk+kk) ----
            u_hi = [
                up.tile([128, gk, d], u_dt, name=f"u_hi{j}", tag=f"u_hi{j}")
                for j in range(G)
            ]
            u_lo = [
                up.tile([128, gk, d], u_dt, name=f"u_lo{j}", tag=f"u_lo{j}")
                for j in range(G)
            ] if use_lo else None
            for j in range(G):
                if l == 0:
                    for ci in range(c):
                        tsl = slice(ci * sgk, (ci + 1) * sgk)
                        nc.sync.dma_start(
                            out=u_hi[j][:, tsl, :], in_=u0_g[ci, j][:, :, 0:d]
                        )
                        if use_lo:
                            nc.sync.dma_start(
                                out=u_lo[j][:, tsl, :],
                                in_=u0_g[ci, j][:, :, d : 2 * d],
                            )
                else:
                    src = ag_outs[rep * (n_layers - 1) + l - 1][j]
                    sv = src.rearrange("(t p) d2 -> p t d2", p=128)
                    nc.sync.dma_start(out=u_hi[j], in_=sv[:, :, 0:d])
                    if use_lo:
                        nc.sync.dma_start(out=u_lo[j], in_=sv[:, :, d : 2 * d])

            # per-layer weight prefetch (bf16)
            wt_t = wtp.tile([128, dh_n, d], BF16, name="wt_t", tag="wt")
            nc.sync.dma_start(
                out=wt_t, in_=wt_dram[l].rearrange("(kc p) o -> p kc o", p=128)
            )

            # skip-connection preload for NEXT layer's input (scaled by dinv)
            nl = l + 1
            skip_sb = None
            if nl in skip_slot and nl < n_layers:
                skip_sb = slabp.tile([128, mq, d], F32, name="skip_sb", tag="skip")
                nc.sync.dma_start(
                    out=skip_sb,
                    in_=skip_dram[skip_slot[nl]].rearrange(
                        "(m p) d2 -> p m d2", p=128
                    ),
                )
                for m in range(mq):
                    nc.vector.tensor_scalar(
                        out=skip_sb[:, m, :],
                        in0=skip_sb[:, m, :],
                        scalar1=dinv_sb[:, m : m + 1],
                        scalar2=None,
                        op0=mybir.AluOpType.mult,
                    )

            # ---- Phase B: mm1  v.T[d, m] accumulate over k ----
            v_sb = [
                work.tile([128, s], BF16, name="v_sb", tag="vsb") for _ in range(dh_n)
            ]
            if not mm1_f8:
                psv = [
                    [pmm1.tile([128, mw], F32, name="psv", tag="pmm1")
                     for _ in range(nmh)]
                    for _ in range(dh_n)
                ]
                for mh in range(nmh):
                    msl = slice(mh * mw, (mh + 1) * mw)
                    for j in range(G):
                        for t in range(gk):
                            ci, kk = divmod(t, sgk)
                            k = ci * (s // 128) + j * sgk + kk
                            rhs = a_sb[:, k, msl]
                            for dh in range(dh_n):
                                nc.tensor.matmul(
                                    psv[dh][mh],
                                    u_hi[j][:, t, dh * 128 : (dh + 1) * 128],
                                    rhs,
                                    start=(j == 0 and t == 0),
                                    stop=(j == G - 1 and t == gk - 1),
                                )
                    for dh in range(dh_n):
                        nc.vector.tensor_copy(v_sb[dh][:, msl], psv[dh][mh])
            elif k_outer and not use_lo:
                ph2 = [
                    [pmm1.tile([128, mw], F32, name="ph2", tag="pmm1")
                     for _ in range(nmh)]
                    for _ in range(dh_n)
                ]
                for j in range(G):
                    for tp_i in range(gk // 2):
                        t0 = 2 * tp_i
                        ci, kk0 = divmod(t0, sgk)
                        k0 = ci * (s // 128) + j * sgk + kk0
                        st_ = (j == 0 and tp_i == 0)
                        sp_ = (j == G - 1 and tp_i == gk // 2 - 1)
                        for dh in range(dh_n):
                            lt = u_hi[j][:, t0 : t0 + 2,
                                         dh * 128 : (dh + 1) * 128]
                            for mh in range(nmh):
                                nc.tensor.matmul(
                                    ph2[dh][mh], lt,
                                    a_sb[:, k0 : k0 + 2,
                                         mh * mw : (mh + 1) * mw],
                                    start=st_, stop=sp_, perf_mode=dr,
                                )
                for mh in range(nmh):
                    for dh in range(dh_n):
                        nc.vector.tensor_copy(
                            v_sb[dh][:, mh * mw : (mh + 1) * mw], ph2[dh][mh]
                        )
            else:
                for mh in range(nmh):
                    msl = slice(mh * mw, (mh + 1) * mw)
                    ph = [pmm1.tile([128, mw], F32, name="ph", tag="pmm1")
                          for _ in range(dh_n)]
                    pl = [pmm1.tile([128, mw], F32, name="pl", tag="pmm1")
                          for _ in range(dh_n)] if use_lo else None
                    for j in range(G):
                        for tp_i in range(gk // 2):
                            t0 = 2 * tp_i
                            ci, kk0 = divmod(t0, sgk)
                            k0 = ci * (s // 128) + j * sgk + kk0
                            rhs = a_sb[:, k0 : k0 + 2, msl]
                            st_ = (j == 0 and tp_i == 0)
                            sp_ = (j == G - 1 and tp_i == gk // 2 - 1)
                            for dh in range(dh_n):
                                dsl = slice(dh * 128, (dh + 1) * 128)
                                nc.tensor.matmul(
                                    ph[dh], u_hi[j][:, t0 : t0 + 2, dsl], rhs,
                                    start=st_, stop=sp_, perf_mode=dr,
                                )
                                if use_lo:
                                    nc.tensor.matmul(
                                        pl[dh], u_lo[j][:, t0 : t0 + 2, dsl], rhs,
                                        start=st_, stop=sp_, perf_mode=dr,
                                    )
                    for dh in range(dh_n):
                        if use_lo:
                            # v = v_hi + v_lo/16 (lo was stored as 16*residual)
                            vtmp = stage.tile([128, mw], BF16, name="vtmp", tag="vtmp")
                            nc.scalar.activation(
                                vtmp, pl[dh], mybir.ActivationFunctionType.Copy,
                                scale=0.0625,
                            )
                            nc.vector.tensor_add(v_sb[dh][:, msl], ph[dh], vtmp)
                        else:
                            nc.vector.tensor_copy(v_sb[dh][:, msl], ph[dh])

            # ---- Phase D/E per m-half: mm2, relu, transpose, u-prep ----
            is_out = l >= n_layers - 3  # layers 4,5,6 emit outputs 0,1,2
            save_skip = l <= 2
            h_nat = None
            if is_out or save_skip:
                h_nat = slabp.tile([128, mq, d], F32, name="h_nat", tag="hnat", bufs=1)
            if l == n_layers - 1:
                # reuse the (now idle) skip/us tags for the final-layer tiles
                h0s = slabp.tile([128, mq, d], F32, name="h0s", tag="skip")
                nc.sync.dma_start(
                    out=h0s, in_=h0s_dram[:, :].rearrange("(m p) d2 -> p m d2", p=128)
                )
                out3 = slabp.tile([128, mq, d], F32, name="out3", tag="us_hi", bufs=1)
            if l < n_layers - 1:
                us = slabp.tile(
                    [128, mq, ud_n * d], u_dt, name="us", tag="us_hi", bufs=1
                )

            hT = [work.tile([128, s], F32, name="hT", tag="hT") for _ in range(dh_n)]
            mq_h = mw // 128  # m-chunks per half
            for mh in range(nmh):
                msl = slice(mh * mw, (mh + 1) * mw)
                pso = [
                    post.tile([128, mw], F32, name="pso", tag="post")
                    for _ in range(dh_n)
                ]
                for dho in range(dh_n):
                    for kin in range(dh_n):
                        nc.tensor.matmul(
                            pso[dho],
                            wt_t[:, kin, dho * 128 : (dho + 1) * 128],
                            v_sb[kin][:, msl],
                            start=(kin == 0),
                            stop=(kin == dh_n - 1),
                        )
                for dho in range(dh_n):
                    nc.vector.tensor_mul(hT[dho][:, msl], pso[dho], dinv_bc[:, msl])
                    nc.scalar.activation(
                        hT[dho][:, msl],
                        hT[dho][:, msl],
                        relu,
                        bias=bias_sb[:, 2 * l + dho : 2 * l + dho + 1],
                    )
                for m in range(mh * mq_h, (mh + 1) * mq_h):
                    tp = post.tile([128, d], F32, name="tp", tag="post")
                    for dh in range(dh_n):
                        nc.tensor.transpose(
                            tp[:, dh * 128 : (dh + 1) * 128],
                            hT[dh][:, m * 128 : (m + 1) * 128],
                            ident,
                        )
                    if l < n_layers - 1:
                        ufp = stage.tile([128, d], F32, name="ufp", tag="ufp")
                        dv = dinv_sb[:, m : m + 1]
                        if skip_sb is not None:
                            nc.vector.scalar_tensor_tensor(
                                out=ufp,
                                in0=tp,
                                scalar=dv,
                                in1=skip_sb[:, m, :],
                                op0=mybir.AluOpType.mult,
                                op1=mybir.AluOpType.add,
                            )
                        else:
                            nc.vector.tensor_scalar(
                                out=ufp,
                                in0=tp,
                                scalar1=dv,
                                scalar2=None,
                                op0=mybir.AluOpType.mult,
                            )
                        if mm1_f8 and f8_single:
                            nc.vector.tensor_scalar(
                                out=us[:, m, 0:d], in0=ufp, scalar1=U_SCALE,
                                scalar2=None, op0=mybir.AluOpType.mult,
                            )
                        else:
                            nc.vector.tensor_copy(us[:, m, 0:d], ufp)
                        if use_lo:
                            ulo = stage.tile([128, d], F32, name="ulo", tag="ulo")
                            nc.vector.tensor_sub(ulo, ufp, us[:, m, 0:d])
                            nc.scalar.activation(
                                us[:, m, d : 2 * d], ulo,
                                mybir.ActivationFunctionType.Copy, scale=16.0,
                            )
                    if h_nat is not None:
                        nc.scalar.copy(h_nat[:, m, :], tp)
                    if l == n_layers - 1:
                        nc.vector.tensor_add(out3[:, m, :], tp, h0s[:, m, :])
                    # group complete -> stage + allgather immediately
                    if l < n_layers - 1 and (m + 1) % gm == 0:
                        j = m // gm
                        agi = ag_ins[j]
                        nc.sync.dma_start(
                            out=agi.rearrange("(mm p) d2 -> p mm d2", p=128),
                            in_=us[:, j * gm : (j + 1) * gm, :],
                        )
                        if ag_tiny:
                            # timing probe: same collective cadence, 4KB payload
                            agti = agt_ins[j]
                            nc.sync.dma_start(
                                out=agti[:, :], in_=us[0:8, j * gm, :]
                            )
                            nc.gpsimd.collective_compute(
                                "AllGather",
                                mybir.AluOpType.bypass,
                                replica_groups=[list(range(c))],
                                ins=[agti.opt()],
                                outs=[agt_outs[rep * (n_layers - 1) + l][j].opt()],
                            )
                        elif not no_ag:
                            nc.gpsimd.collective_compute(
                                "AllGather",
                                mybir.AluOpType.bypass,
                                replica_groups=[list(range(c))],
                                ins=[agi.opt()],
                                outs=[ag_outs[rep * (n_layers - 1) + l][j].opt()],
                            )
                        else:
                            # timing-only mode: fake the gather with a local
                            # DMA of the slab into own block of the output
                            nc.sync.dma_start(
                                out=ag_outs[rep * (n_layers - 1) + l][j][
                                    0 : s // G, :
                                ],
                                in_=agi[:, :],
                            )

            # ---- Phase F: DMAs out ----
            if save_skip:
                nc.sync.dma_start(
                    out=skip_dram[l].rearrange("(m p) d2 -> p m d2", p=128),
                    in_=h_nat,
                )
            if is_out:
                nc.sync.dma_start(
                    out=out_dram[l - (n_layers - 3)].rearrange(
                        "(m p) d2 -> p m d2", p=128
                    ),
                    in_=h_nat,
                )
            if l == n_layers - 1:
                nc.sync.dma_start(
                    out=out_dram[3].rearrange("(m p) d2 -> p m d2", p=128), in_=out3
                )

    nc.compile()
    return nc


try:
    import ml_dtypes

    ml_bf16 = ml_dtypes.bfloat16
    ml_f8 = ml_dtypes.float8_e4m3fn
except ImportError:  # pragma: no cover
    import jax.numpy as jnp

    ml_bf16 = jnp.bfloat16
    ml_f8 = jnp.float8_e4m3fn


def prep_inputs(g, h, W_down, b_down, W_bottom, b_bottom, W_up, b_up, c=C,
                mm1_f8=MM1_F8, f8_single=F8_SINGLE):
    """Host-side sharding + layout prep. Returns per-core input maps."""
    n = g.shape[0]
    s = n // c
    d = h.shape[1]
    g = np.asarray(g, np.float32)
    h = np.asarray(h, np.float32)
    deg = g.sum(axis=1) + 1.0
    dinv = (1.0 / np.sqrt(deg)).astype(np.float32)

    u0 = (h * dinv[:, None]).astype(np.float32)
    if mm1_f8 and f8_single:
        u0_packed = np.asarray((u0 * 32.0).astype(ml_f8))  # [n, d] scaled fp8
    elif mm1_f8:
        u0_hi = u0.astype(ml_f8)
        u0_lo = ((u0 - u0_hi.astype(np.float32)) * 16.0).astype(ml_f8)
        u0_packed = np.concatenate([np.asarray(u0_hi), np.asarray(u0_lo)], axis=1)
    else:
        u0_packed = np.asarray(u0.astype(ml_bf16))  # [n, d] bf16

    Ws = [W_down[0], W_down[1], W_down[2], W_bottom, W_up[0], W_up[1], W_up[2]]
    bs = [b_down[0], b_down[1], b_down[2], b_bottom, b_up[0], b_up[1], b_up[2]]
    wt = np.stack(
        [np.ascontiguousarray(np.asarray(W, np.float32).T).astype(ml_bf16) for W in Ws]
    )
    nl = len(Ws)
    bias_t = np.zeros((128, 2 * nl), np.float32)
    for li, b in enumerate(bs):
        b = np.asarray(b, np.float32)
        for dh in range(d // 128):
            bias_t[:, 2 * li + dh] = b[dh * 128 : (dh + 1) * 128]

    in_maps = []
    for ci in range(c):
        sl = slice(ci * s, (ci + 1) * s)
        a_slab = np.ascontiguousarray(g[:, sl])
        idx = np.arange(s)
        a_slab[ci * s + idx, idx] += 1.0  # fold self-loops into the slab
        dinv_slab = dinv[sl].reshape(s // 128, 128).T.copy()  # [128, mq]
        dsc = dinv / 32.0 if (mm1_f8 and f8_single) else dinv
        dinv_bcast = np.broadcast_to(dsc[sl][None, :], (128, s)).copy()
        in_maps.append(
            dict(
                a_slab=a_slab,
                u0=u0_packed,
                h0_slab=np.ascontiguousarray(h[sl]),
                dinv_slab=dinv_slab,
                dinv_bcast=dinv_bcast,
                wt=wt,
                bias_t=bias_t,
            )
        )
    return in_maps


_NC_CACHE = {}


def kernel(g, h, W_down, b_down, W_bottom, b_bottom, W_up, b_up):
    key = "full"
    if key not in _NC_CACHE:
        _NC_CACHE[key] = build_nc()
    nc = _NC_CACHE[key]
    in_maps = prep_inputs(g, h, W_down, b_down, W_bottom, b_bottom, W_up, b_up)
    res = run_bass_kernel_spmd(nc, in_maps, list(range(C)))
    outs = [np.asarray(r["out"]).reshape(4, S, D) for r in res.results]
    full = np.concatenate(outs, axis=1)  # [4, N, D]
    return full.astype(np.float32)


if __name__ == "__main__":
    import reference

    inputs = reference.setup_inputs()
    inputs = {k: np.asarray(v) for k, v in inputs.items()}
    out = kernel(**inputs)
    exp = np.asarray(reference.reference(**reference.setup_inputs()))
    err = np.abs(out - exp).max() / (np.abs(exp).max() + 1e-30)
    rel = np.linalg.norm(out - exp) / (np.linalg.norm(exp) + 1e-30)
    print("max-scaled err:", err, "rel l2:", rel)


# revision 16
# speedup vs baseline: 4.4384x; 1.7520x over previous
"""GraphUnetNoPool (7-layer GCN U-net, no pooling) on 8 trn2 NeuronCores.

Math: gn = D^-1/2 (g+I) D^-1/2;  layer: h' = relu(gn @ h @ W.T + b)
Rewrite: u = dinv*h;  v = (g+I) @ u;  h' = relu((dinv*v) @ W.T + b)
  =>  per-core row-slab m:  v.T[d, m] = sum_k u[k, d] * A[k, m]  (A symmetric:
      column slab of A == transposed row slab, so lhsT = u natural layout and
      rhs = A[:, slab] streams naturally from DRAM rows).
Sharding: 1D row-parallel. Core c owns rows [c*S, (c+1)*S).

v2: mm1 in single bf16 (u rounded to bf16; A exact in fp8), A fully resident
in SBUF as fp8, mm2/weights in bf16, per-layer AllGather of u split into G
groups, each issued as soon as its row block is ready so the collective
overlaps the next layer's mm1 (which consumes gathered groups in order).
v3 (mm1_f8): u as fp8 hi + 16*lo pair, mm1 via DoubleRow fp8 matmuls (2
k-chunks per instruction), v = v_hi + v_lo/16.
"""

import numpy as np
from contextlib import ExitStack

import concourse.bass as bass
import concourse.tile as tile
from concourse import bacc, mybir
from concourse.bass_utils import run_bass_kernel_spmd
from concourse.masks import make_identity

F32 = mybir.dt.float32
BF16 = mybir.dt.bfloat16
F8 = mybir.dt.float8e4

N, D, C, L = 8192, 256, 8, 7
S = N // C            # 1024 rows per core
KC = N // 128         # 64 k-chunks
MQ = S // 128         # 8 m-chunks per slab
G_AG = 4              # allgather split groups
A_DT = F8             # resident adjacency dtype ({0,1,2} exact in fp8e4)
MM1_F8 = True         # fp8 DoubleRow mm1 (u as hi + 16*lo fp8 pair)
F8_SINGLE = True      # single scaled-e4m3 plane (u*32); halves wire + mm1
U_SCALE = 32.0
K_OUTER = False       # mm1 k-outer: reuse each stationary for both m-halves


def build_nc(n=N, d=D, c=C, n_layers=L, repeat=1, g_ag=G_AG, a_dt=A_DT,
             mm1_f8=MM1_F8, no_ag=False, ag_tiny=False, f8_single=F8_SINGLE,
             k_outer=K_OUTER):
    s = n // c
    kc = n // 128
    mq = s // 128
    nmh = s // 512 if s >= 512 else 1   # moving halves of 512 (psum bank)
    mw = min(s, 512)                    # moving width
    dh_n = d // 128                     # d chunks (2 for d=256)
    G = g_ag
    sgk = (s // G) // 128               # k-chunks per (core, group)
    gm = mq // G                        # m-chunks per group
    gk = kc // G                        # k-chunks per group (all cores)
    assert d % 128 == 0 and s % (128 * G) == 0
    if mm1_f8:
        assert a_dt == F8 and sgk % 2 == 0
    u_dt = F8 if mm1_f8 else BF16
    use_lo = mm1_f8 and not f8_single
    ud_n = 2 if use_lo else 1           # u planes (hi [, scaled lo])
    dr = mybir.MatmulPerfMode.DoubleRow

    nc = bacc.Bacc("TRN2", target_bir_lowering=False, debug=False, num_devices=c)

    a_dram = nc.dram_tensor("a_slab", [n, s], F32, kind="ExternalInput")
    u0_dram = nc.dram_tensor("u0", [n, ud_n * d], u_dt, kind="ExternalInput")
    h0s_dram = nc.dram_tensor("h0_slab", [s, d], F32, kind="ExternalInput")
    dslab_dram = nc.dram_tensor("dinv_slab", [128, mq], F32, kind="ExternalInput")
    dbc_dram = nc.dram_tensor("dinv_bcast", [128, s], F32, kind="ExternalInput")
    wt_dram = nc.dram_tensor("wt", [n_layers, d, d], BF16, kind="ExternalInput")
    bias_dram = nc.dram_tensor("bias_t", [128, 2 * n_layers], F32, kind="ExternalInput")
    out_dram = nc.dram_tensor("out", [4, s, d], F32, kind="ExternalOutput")

    with ExitStack() as ctx:
        tc = ctx.enter_context(tile.TileContext(nc))
        dram = ctx.enter_context(tc.tile_pool(name="dram", bufs=1, space="DRAM"))
        res = ctx.enter_context(tc.tile_pool(name="res", bufs=1))
        stage = ctx.enter_context(tc.tile_pool(name="stage", bufs=2))
        up = ctx.enter_context(tc.tile_pool(name="up", bufs=2))
        wtp = ctx.enter_context(tc.tile_pool(name="wtp", bufs=2))
        work = ctx.enter_context(tc.tile_pool(name="work", bufs=2))
        slabp = ctx.enter_context(tc.tile_pool(name="slabp", bufs=2))
        pmm1 = ctx.enter_context(tc.tile_pool(name="pmm1", bufs=4, space="PSUM"))
        post = ctx.enter_context(tc.tile_pool(name="post", bufs=4, space="PSUM"))

        # ---- persistent DRAM scratch ----
        ag_ins = [
            dram.tile([s // G, ud_n * d], u_dt, name=f"ag_in{j}", tag=f"ag_in{j}",
                      bufs=2)
            for j in range(G)
        ]
        if ag_tiny:
            agt_ins = [
                dram.tile([8, ud_n * d], u_dt, name=f"agt_in{j}", tag=f"agt_in{j}",
                          bufs=2)
                for j in range(G)
            ]
            agt_outs = [
                [
                    dram.tile(
                        [8 * c, ud_n * d], u_dt, name=f"agt_out{i}_{j}",
                        tag=f"agt_out{i}_{j}", addr_space="Shared",
                    )
                    for j in range(G)
                ]
                for i in range((n_layers - 1) * repeat)
            ]
        ag_outs = [
            [
                dram.tile(
                    [(n // G), ud_n * d], u_dt, name=f"ag_out{i}_{j}",
                    tag=f"ag_out{i}_{j}", addr_space="Shared",
                )
                for j in range(G)
            ]
            for i in range((n_layers - 1) * repeat)
        ]
        skip_dram = dram.tile([3, s, d], F32, name="skip_dram")

        # ---- persistent SBUF ----
        a_sb = res.tile([128, kc, s], a_dt, name="a_sb")
        dinv_sb = res.tile([128, mq], F32, name="dinv_sb")
        dinv_bc = res.tile([128, s], F32, name="dinv_bc")
        bias_sb = res.tile([128, 2 * n_layers], F32, name="bias_sb")
        ident = res.tile([128, 128], F32, name="ident")

        make_identity(nc, ident)
        nc.sync.dma_start(out=dinv_sb, in_=dslab_dram[:, :])
        nc.sync.dma_start(out=dinv_bc, in_=dbc_dram[:, :])
        nc.sync.dma_start(out=bias_sb, in_=bias_dram[:, :])

        # ---- startup: load A column-slab, cast to a_dt, fully resident ----
        for k in range(kc):
            st = stage.tile([128, s], F32, name="st", tag="stage")
            nc.sync.dma_start(out=st, in_=a_dram[k * 128 : (k + 1) * 128, :])
            nc.vector.tensor_copy(a_sb[:, k, :], st)

        relu = mybir.ActivationFunctionType.Relu
        skip_slot = {4: 2, 5: 1, 6: 0}  # up-layer l uses skip h_{...} slot

        # u0 grouped view: [ci, g, p, kk, d-planes] (per-ci DMAs at layer 0)
        u0_g = u0_dram[:, :].rearrange(
            "(ci g kk p) d2 -> ci g p kk d2", g=G, kk=sgk, p=128
        )

        for rep_l in range(n_layers * repeat):
            rep, l = divmod(rep_l, n_layers)
            # ---- Phase A: load U per gather group (flat t = ci*sgk+kk) ----
            u_hi = [
                up.tile([128, gk, d], u_dt, name=f"u_hi{j}", tag=f"u_hi{j}")
                for j in range(G)
            ]
            u_lo = [
                up.tile([128, gk, d], u_dt, name=f"u_lo{j}", tag=f"u_lo{j}")
                for j in range(G)
            ] if use_lo else None
            for j in range(G):
                if l == 0:
                    for ci in range(c):
                        tsl = slice(ci * sgk, (ci + 1) * sgk)
                        nc.sync.dma_start(
                            out=u_hi[j][:, tsl, :], in_=u0_g[ci, j][:, :, 0:d]
                        )
                        if use_lo:
                            nc.sync.dma_start(
                                out=u_lo[j][:, tsl, :],
                                in_=u0_g[ci, j][:, :, d : 2 * d],
                            )
                else:
                    src = ag_outs[rep * (n_layers - 1) + l - 1][j]
                    sv = src.rearrange("(t p) d2 -> p t d2", p=128)
                    nc.sync.dma_start(out=u_hi[j], in_=sv[:, :, 0:d])
                    if use_lo:
                        nc.sync.dma_start(out=u_lo[j], in_=sv[:, :, d : 2 * d])

            # per-layer weight prefetch (bf16)
            wt_t = wtp.tile([128, dh_n, d], BF16, name="wt_t", tag="wt")
            nc.sync.dma_start(
                out=wt_t, in_=wt_dram[l].rearrange("(kc p) o -> p kc o", p=128)
            )

            # skip-connection preload for NEXT layer's input (scaled by dinv)
            nl = l + 1
            skip_sb = None
            if nl in skip_slot and nl < n_layers:
                skip_sb = slabp.tile([128, mq, d], F32, name="skip_sb", tag="skip")
                nc.sync.dma_start(
                    out=skip_sb,
                    in_=skip_dram[skip_slot[nl]].rearrange(
                        "(m p) d2 -> p m d2", p=128
                    ),
                )
                for m in range(mq):
                    nc.vector.tensor_scalar(
                        out=skip_sb[:, m, :],
                        in0=skip_sb[:, m, :],
                        scalar1=dinv_sb[:, m : m + 1],
                        scalar2=None,
                        op0=mybir.AluOpType.mult,
                    )

            # ---- Phase B: mm1  v.T[d, m] accumulate over k ----
            v_sb = [
                work.tile([128, s], BF16, name="v_sb", tag="vsb") for _ in range(dh_n)
            ]
            if not mm1_f8:
                psv = [
                    [pmm1.tile([128, mw], F32, name="psv", tag="pmm1")
                     for _ in range(nmh)]
                    for _ in range(dh_n)
                ]
                for mh in range(nmh):
                    msl = slice(mh * mw, (mh + 1) * mw)
                    for j in range(G):
                        for t in range(gk):
                            ci, kk = divmod(t, sgk)
                            k = ci * (s // 128) + j * sgk + kk
                            rhs = a_sb[:, k, msl]
                            for dh in range(dh_n):
                                nc.tensor.matmul(
                                    psv[dh][mh],
                                    u_hi[j][:, t, dh * 128 : (dh + 1) * 128],
                                    rhs,
                                    start=(j == 0 and t == 0),
                                    stop=(j == G - 1 and t == gk - 1),
                                )
                    for dh in range(dh_n):
                        nc.vector.tensor_copy(v_sb[dh][:, msl], psv[dh][mh])
            elif k_outer and not use_lo:
                ph2 = [
                    [pmm1.tile([128, mw], F32, name="ph2", tag="pmm1")
                     for _ in range(nmh)]
                    for _ in range(dh_n)
                ]
                for j in range(G):
                    for tp_i in range(gk // 2):
                        t0 = 2 * tp_i
                        ci, kk0 = divmod(t0, sgk)
                        k0 = ci * (s // 128) + j * sgk + kk0
                        st_ = (j == 0 and tp_i == 0)
                        sp_ = (j == G - 1 and tp_i == gk // 2 - 1)
                        for dh in range(dh_n):
                            lt = u_hi[j][:, t0 : t0 + 2,
                                         dh * 128 : (dh + 1) * 128]
                            for mh in range(nmh):
                                nc.tensor.matmul(
                                    ph2[dh][mh], lt,
                                    a_sb[:, k0 : k0 + 2,
                                         mh * mw : (mh + 1) * mw],
                                    start=st_, stop=sp_, perf_mode=dr,
                                )
                for mh in range(nmh):
                    for dh in range(dh_n):
                        nc.vector.tensor_copy(
                            v_sb[dh][:, mh * mw : (mh + 1) * mw], ph2[dh][mh]
                        )
            else:
                for mh in range(nmh):
                    msl = slice(mh * mw, (mh + 1) * mw)
                    ph = [pmm1.tile([128, mw], F32, name="ph", tag="pmm1")
                          for _ in range(dh_n)]
                    pl = [pmm1.tile([128, mw], F32, name="pl", tag="pmm1")
                          for _ in range(dh_n)] if use_lo else None
                    for j in range(G):
                        for tp_i in range(gk // 2):
                            t0 = 2 * tp_i
                            ci, kk0 = divmod(t0, sgk)
                            k0 = ci * (s // 128) + j * sgk + kk0
                            rhs = a_sb[:, k0 : k0 + 2, msl]
                            st_ = (j == 0 and tp_i == 0)
                            sp_ = (j == G - 1 and tp_i == gk // 2 - 1)
                            for dh in range(dh_n):
                                dsl = slice(dh * 128, (dh + 1) * 128)
                                nc.tensor.matmul(
                                    ph[dh], u_hi[j][:, t0 : t0 + 2, dsl], rhs,
                                    start=st_, stop=sp_, perf_mode=dr,
                                )
                                if use_lo:
                                    nc.tensor.matmul(
                                        pl[dh], u_lo[j][:, t0 : t0 + 2, dsl], rhs,
                                        start=st_, stop=sp_, perf_mode=dr,
                                    )
                    for dh in range(dh_n):
                        if use_lo:
                            # v = v_hi + v_lo/16 (lo was stored as 16*residual)
                            vtmp = stage.tile([128, mw], BF16, name="vtmp", tag="vtmp")
                            nc.scalar.activation(
                                vtmp, pl[dh], mybir.ActivationFunctionType.Copy,
                                scale=0.0625,
                            )
                            nc.vector.tensor_add(v_sb[dh][:, msl], ph[dh], vtmp)
                        else:
                            nc.vector.tensor_copy(v_sb[dh][:, msl], ph[dh])

            # ---- Phase D/E per m-half: mm2, relu, transpose, u-prep ----
            is_out = l >= n_layers - 3  # layers 4,5,6 emit outputs 0,1,2
            save_skip = l <= 2
            h_nat = None
            if is_out or save_skip:
                h_nat = slabp.tile([128, mq, d], F32, name="h_nat", tag="hnat", bufs=1)
            if l == n_layers - 1:
                # reuse the (now idle) skip/us tags for the final-layer tiles
                h0s = slabp.tile([128, mq, d], F32, name="h0s", tag="skip")
                nc.sync.dma_start(
                    out=h0s, in_=h0s_dram[:, :].rearrange("(m p) d2 -> p m d2", p=128)
                )
                out3 = slabp.tile([128, mq, d], F32, name="out3", tag="us_hi", bufs=1)
            if l < n_layers - 1:
                us = slabp.tile(
                    [128, mq, ud_n * d], u_dt, name="us", tag="us_hi", bufs=1
                )

            hT = [work.tile([128, s], F32, name="hT", tag="hT") for _ in range(dh_n)]
            mq_h = mw // 128  # m-chunks per half
            for mh in range(nmh):
                msl = slice(mh * mw, (mh + 1) * mw)
                pso = [
                    post.tile([128, mw], F32, name="pso", tag="post")
                    for _ in range(dh_n)
                ]
                for dho in range(dh_n):
                    for kin in range(dh_n):
                        nc.tensor.matmul(
                            pso[dho],
                            wt_t[:, kin, dho * 128 : (dho + 1) * 128],
                            v_sb[kin][:, msl],
                            start=(kin == 0),
                            stop=(kin == dh_n - 1),
                        )
                for dho in range(dh_n):
                    nc.vector.tensor_mul(hT[dho][:, msl], pso[dho], dinv_bc[:, msl])
                    nc.scalar.activation(
                        hT[dho][:, msl],
                        hT[dho][:, msl],
                        relu,
                        bias=bias_sb[:, 2 * l + dho : 2 * l + dho + 1],
                    )
                for m in range(mh * mq_h, (mh + 1) * mq_h):
                    tp = post.tile([128, d], F32, name="tp", tag="post")
                    for dh in range(dh_n):
                        nc.tensor.transpose(
                            tp[:, dh * 128 : (dh + 1) * 128],
                            hT[dh][:, m * 128 : (m + 1) * 128],
                            ident,
                        )
                    if l < n_layers - 1:
                        ufp = stage.tile([128, d], F32, name="ufp", tag="ufp")
                        dv = dinv_sb[:, m : m + 1]
                        if skip_sb is not None:
                            nc.vector.scalar_tensor_tensor(
                                out=ufp,
                                in0=tp,
                                scalar=dv,
                                in1=skip_sb[:, m, :],
                                op0=mybir.AluOpType.mult,
                                op1=mybir.AluOpType.add,
                            )
                        else:
                            nc.vector.tensor_scalar(
                                out=ufp,
                                in0=tp,
                                scalar1=dv,
                                scalar2=None,
                                op0=mybir.AluOpType.mult,
                            )
                        if mm1_f8 and f8_single:
                            nc.vector.tensor_scalar(
                                out=us[:, m, 0:d], in0=ufp, scalar1=U_SCALE,
                                scalar2=None, op0=mybir.AluOpType.mult,
                            )
                        else:
                            nc.vector.tensor_copy(us[:, m, 0:d], ufp)
                        if use_lo:
                            ulo = stage.tile([128, d], F32, name="ulo", tag="ulo")
                            nc.vector.tensor_sub(ulo, ufp, us[:, m, 0:d])
                            nc.scalar.activation(
                                us[:, m, d : 2 * d], ulo,
                                mybir.ActivationFunctionType.Copy, scale=16.0,
                            )
                    if h_nat is not None:
                        nc.scalar.copy(h_nat[:, m, :], tp)
                    if l == n_layers - 1:
                        nc.vector.tensor_add(out3[:, m, :], tp, h0s[:, m, :])
                    # group complete -> stage + allgather immediately
                    if l < n_layers - 1 and (m + 1) % gm == 0:
                        j = m // gm
                        agi = ag_ins[j]
                        nc.sync.dma_start(
                            out=agi.rearrange("(mm p) d2 -> p mm d2", p=128),
                            in_=us[:, j * gm : (j + 1) * gm, :],
                        )
                        if ag_tiny:
                            # timing probe: same collective cadence, 4KB payload
                            agti = agt_ins[j]
                            nc.sync.dma_start(
                                out=agti[:, :], in_=us[0:8, j * gm, :]
                            )
                            nc.gpsimd.collective_compute(
                                "AllGather",
                                mybir.AluOpType.bypass,
                                replica_groups=[list(range(c))],
                                ins=[agti.opt()],
                                outs=[agt_outs[rep * (n_layers - 1) + l][j].opt()],
                            )
                        elif not no_ag:
                            nc.gpsimd.collective_compute(
                                "AllGather",
                                mybir.AluOpType.bypass,
                                replica_groups=[list(range(c))],
                                ins=[agi.opt()],
                                outs=[ag_outs[rep * (n_layers - 1) + l][j].opt()],
                            )
                        else:
                            # timing-only mode: fake the gather with a local
                            # DMA of the slab into own block of the output
                            nc.sync.dma_start(
                                out=ag_outs[rep * (n_layers - 1) + l][j][
                                    0 : s // G, :
                                ],
                                in_=agi[:, :],
                            )

            # ---- Phase F: DMAs out ----
            if save_skip:
                nc.sync.dma_start(
                    out=skip_dram[l].rearrange("(m p) d2 -> p m d2", p=128),
                    in_=h_nat,
                )
            if is_out:
                nc.sync.dma_start(
                    out=out_dram[l - (n_layers - 3)].rearrange(
                        "(m p) d2 -> p m d2", p=128
                    ),
                    in_=h_nat,
                )
            if l == n_layers - 1:
                nc.sync.dma_start(
                    out=out_dram[3].rearrange("(m p) d2 -> p m d2", p=128), in_=out3
                )

    nc.compile()
    return nc


try:
    import ml_dtypes

    ml_bf16 = ml_dtypes.bfloat16
    ml_f8 = ml_dtypes.float8_e4m3fn
except ImportError:  # pragma: no cover
    import jax.numpy as jnp

    ml_bf16 = jnp.bfloat16
    ml_f8 = jnp.float8_e4m3fn


def prep_inputs(g, h, W_down, b_down, W_bottom, b_bottom, W_up, b_up, c=C,
                mm1_f8=MM1_F8, f8_single=F8_SINGLE):
    """Host-side sharding + layout prep. Returns per-core input maps."""
    n = g.shape[0]
    s = n // c
    d = h.shape[1]
    g = np.asarray(g, np.float32)
    h = np.asarray(h, np.float32)
    deg = g.sum(axis=1) + 1.0
    dinv = (1.0 / np.sqrt(deg)).astype(np.float32)

    u0 = (h * dinv[:, None]).astype(np.float32)
    if mm1_f8 and f8_single:
        u0_packed = np.asarray((u0 * 32.0).astype(ml_f8))  # [n, d] scaled fp8
    elif mm1_f8:
        u0_hi = u0.astype(ml_f8)
        u0_lo = ((u0 - u0_hi.astype(np.float32)) * 16.0).astype(ml_f8)
        u0_packed = np.concatenate([np.asarray(u0_hi), np.asarray(u0_lo)], axis=1)
    else:
        u0_packed = np.asarray(u0.astype(ml_bf16))  # [n, d] bf16

    Ws = [W_down[0], W_down[1], W_down[2], W_bottom, W_up[0], W_up[1], W_up[2]]
    bs = [b_down[0], b_down[1], b_down[2], b_bottom, b_up[0], b_up[1], b_up[2]]
    wt = np.stack(
        [np.ascontiguousarray(np.asarray(W, np.float32).T).astype(ml_bf16) for W in Ws]
    )
    nl = len(Ws)
    bias_t = np.zeros((128, 2 * nl), np.float32)
    for li, b in enumerate(bs):
        b = np.asarray(b, np.float32)
        for dh in range(d // 128):
            bias_t[:, 2 * li + dh] = b[dh * 128 : (dh + 1) * 128]

    in_maps = []
    for ci in range(c):
        sl = slice(ci * s, (ci + 1) * s)
        a_slab = np.ascontiguousarray(g[:, sl])
        idx = np.arange(s)
        a_slab[ci * s + idx, idx] += 1.0  # fold self-loops into the slab
        dinv_slab = dinv[sl].reshape(s // 128, 128).T.copy()  # [128, mq]
        dsc = dinv / 32.0 if (mm1_f8 and f8_single) else dinv
        dinv_bcast = np.broadcast_to(dsc[sl][None, :], (128, s)).copy()
        in_maps.append(
            dict(
                a_slab=a_slab,
                u0=u0_packed,
                h0_slab=np.ascontiguousarray(h[sl]),
                dinv_slab=dinv_slab,
                dinv_bcast=dinv_bcast,
                wt=wt,
                bias_t=bias_t,
            )
        )
    return in_maps


_NC_CACHE = {}


def kernel(g, h, W_down, b_down, W_bottom, b_bottom, W_up, b_up):
    key = "full"
    if key not in _NC_CACHE:
        _NC_CACHE[key] = build_nc()
    nc = _NC_CACHE[key]
    in_maps = prep_inputs(g, h, W_down, b_down, W_bottom, b_bottom, W_up, b_up)
    res = run_bass_kernel_spmd(nc, in_maps, list(range(C)))
    outs = [np.asarray(r["out"]).reshape(4, S, D) for r in res.results]
    full = np.concatenate(outs, axis=1)  # [4, N, D]
    return full.astype(np.float32)


if __name__ == "__main__":
    import reference

    inputs = reference.setup_inputs()
    inputs = {k: np.asarray(v) for k, v in inputs.items()}
    out = kernel(**inputs)
    exp = np.asarray(reference.reference(**reference.setup_inputs()))
    err = np.abs(out - exp).max() / (np.abs(exp).max() + 1e-30)
    rel = np.linalg.norm(out - exp) / (np.linalg.norm(exp) + 1e-30)
    print("max-scaled err:", err, "rel l2:", rel)
